# revision 10
# baseline (speedup 1.0000x reference)
"""Multi-head attention (B=2, S=2048, EMB=1024, H=16, hd=64) on 8 TRN2 cores.

Sharding: core c -> batch b = c//4, head-group g = c%4 (4 heads, 256 emb dims).
Per core (all matmuls bf16: full-rate 1 cyc/row streaming):
  A) Q^T = Wq_g @ x_b^T   [256, 2048]   (transposed layout, contraction on emb)
     K^T = Wk_g @ x_b^T   [256, 2048]
     V   = x_b @ Wv_g^T   [2048, 256]   (natural layout, +ones column per head)
  B) per head h: S^T[k,q] = K_h @ Q_h^T (16 k-tiles x [128, 2048] psum)
     P^T = exp(S^T/8): split between ACT (native Exp) and DVE (custom 2-op
     chain: deg-4 poly p~exp(s/64) then p^8) so the softmax isn't ACT-bound;
     U_aug[65, 2048] += [V_h|1].T @ P^T  (row 64 = softmax sums)
  C) r = 1/sums (DVE recip approx); broadcast r over 64 partitions via
     indicator matmul; O^T = U^T * r on GpSimd (written over the Q^T buffer)
  D) y = O @ Wo_g^T partial [2048, 1024]; host sums the 4 head-group partials.
"""
import numpy as np

import concourse.bass as bass
import concourse.tile as tile
from concourse import bacc, mybir
from concourse.bass_utils import run_bass_kernel_spmd

import os

F32 = mybir.dt.float32
F32R = mybir.dt.float32r
BF16 = mybir.dt.bfloat16
FP16 = mybir.dt.float16
# matmul dtype: bf16 (1 cyc/row, ~6e-3) | fp16 (2 cyc/row, ~1e-3) | f32r
MM_DT_NAME = os.environ.get("MM_DT", "bf16")
MM = {"f32r": F32R, "bf16": BF16, "fp16": FP16}[MM_DT_NAME]
IN_DT = {"f32r": F32, "bf16": BF16, "fp16": FP16}[MM_DT_NAME]
EXP = mybir.ActivationFunctionType.Exp
MULT = mybir.AluOpType.mult

EMB = 1024
S = 2048
B = 2
HG = 4           # heads per core
HD = 64
CHD = HG * HD    # 256 emb dims per core
ET = EMB // 128  # 8 e-tiles
NT = S // 128    # 16 s/k-tiles
QB = 512
NQB = S // QB    # 4

_NC = None

# ---- custom DVE exp: p(s) ~ exp(s/64) (deg-4, a0=1), then p^8 -------------
# minimax-with-a0=1 coefficients for exp(u) on u in [-0.8, 0.8], folded with
# the 1/64 argument scale (b_i = a_i / 64^i).
_A = (0.99930331, 0.49979974, 0.17207327, 0.04243063)
EXP_B1 = _A[0] / 64.0
EXP_B2 = _A[1] / 64.0 ** 2
EXP_B3 = _A[2] / 64.0 ** 3
EXP_B4 = _A[3] / 64.0 ** 4

# B-stream exp tile engine per t (A-stream always ACT):
#   act = ACT native exp; dve = DVE poly+pow8; dvp = DVE poly, Pool squarings
B_SCHED = ("dvp", "dve", "act", "dvp", "dve", "dve", "dvp", "act",
           "dve", "dvp", "dve", "dve", "dvp", "act", "act", "dvp")


def _register_dve_op(name, spec, subdim=False):
    import concourse.dve_ops as dvo
    from concourse.dve_uop import DveOpSpec
    from concourse.dve_spec import lower
    from concourse.dve_spec import _has_src1 as has_src1

    for op in dvo.OPS:
        if op.name == name:
            return op
    opcode = dvo._CUSTOM_DVE_ROW_BASE + len(dvo.OPS)
    assert opcode < 0x20
    dvo._SUB_OPCODE_FOR_NAME[name] = opcode
    shas = {}
    for ver in ("v3", "v4"):
        tmp = DveOpSpec(name=name, opcode=opcode, uops=lower(spec, ver=ver),
                        rd1_en=has_src1(spec))
        shas[ver] = tmp.sha(ver)
    op = dvo.DveOp(name, spec, subdim=subdim, uops_sha=shas)
    dvo.OPS.append(op)
    dvo.CUSTOM_DVE_SPECS[name] = spec
    return op


def _make_exp_ops():
    from concourse.dve_spec import (
        Spec, Src0, C0, C1, C2, C3, One, sq, _spill_c3_to_src1,
    )

    u = Src0
    body = _spill_c3_to_src1(((((u * C0 + C1) * u + C2) * u + C3) * u) + One)

    def _ref_poly(in0, in1, s0, s1, imm2):
        return ((((in0 * s0 + s1) * in0 + imm2) * in0 + in1) * in0
                + np.float32(1.0)).astype(np.float32)

    poly = _register_dve_op("EXP_POLY4_ANT", Spec(body=body, reference=_ref_poly))

    def _ref_pow8(in0, in1, s0, s1, imm2):
        q = (in0 * in0).astype(np.float32)
        q = (q * q).astype(np.float32)
        return (q * q).astype(np.float32)

    pow8 = _register_dve_op("POW8_ANT",
                            Spec(body=sq(sq(sq(Src0))), reference=_ref_pow8))
    return poly, pow8


EXP_POLY4, POW8 = _make_exp_ops()


def _mm(ap):
    """View a dram input AP with the matmul dtype (bitcast only for f32r)."""
    return ap.bitcast(F32R) if MM == F32R else ap


def _build():
    nc = bacc.Bacc("TRN2", target_bir_lowering=False, debug=False)
    xq_t = nc.dram_tensor("xq_t", [EMB, S], IN_DT, kind="ExternalInput").ap()
    xk_t = nc.dram_tensor("xk_t", [EMB, S], IN_DT, kind="ExternalInput").ap()
    xv_t = nc.dram_tensor("xv_t", [EMB, S], IN_DT, kind="ExternalInput").ap()
    wq_t = nc.dram_tensor("wq_t", [EMB, CHD], IN_DT, kind="ExternalInput").ap()
    wk_t = nc.dram_tensor("wk_t", [EMB, CHD], IN_DT, kind="ExternalInput").ap()
    wv_t = nc.dram_tensor("wv_t", [EMB, CHD], IN_DT, kind="ExternalInput").ap()
    wo_t = nc.dram_tensor("wo_t", [CHD, EMB], IN_DT, kind="ExternalInput").ap()
    y = nc.dram_tensor("y", [S, EMB], F32, kind="ExternalOutput").ap()

    with tile.TileContext(nc) as tc:
        with tc.tile_pool(name="const", bufs=1) as cpool, \
             tc.tile_pool(name="wqk", bufs=2) as wpool, \
             tc.tile_pool(name="big", bufs=1) as big, \
             tc.tile_pool(name="usb", bufs=4) as usb, \
             tc.tile_pool(name="xp", bufs=8) as xp, \
             tc.tile_pool(name="pt", bufs=2) as ptp, \
             tc.tile_pool(name="esc", bufs=3) as escp, \
             tc.tile_pool(name="psq", bufs=2) as psqp, \
             tc.tile_pool(name="yp", bufs=2) as ypool, \
             tc.tile_pool(name="rp", bufs=2) as rpool, \
             tc.tile_pool(name="rd", bufs=4, space="DRAM") as rdram:

            # ---- static weights (wo DMA deferred past phase A) ----
            wo_sb = cpool.tile([128, 2, EMB], MM, name="wo_sb")
            b1c = cpool.tile([128, 1], F32, name="b1c")
            nc.vector.memset(b1c[:], EXP_B1)

            qT = big.tile([128, 2, S], MM, name="qT")     # later reused as O^T
            kT = big.tile([128, 2, S], MM, name="kT")
            v_sb = big.tile([128, NT, HG * (HD + 1)], MM, name="v_sb")
            if MM == F32R:
                nc.vector.memset(v_sb[:].bitcast(F32), 1.0)
            else:
                nc.vector.memset(v_sb[:], 1.0)     # ones cols survive

            # ---- phase A: projections ----
            warm0 = cpool.tile([128, QB], MM, name="warm0")
            nc.vector.memset(warm0[:], 1.0)
            with tc.tile_pool(name="psA", bufs=8, space="PSUM") as psA:
                # Q^T and K^T: out[m, q] accumulated over e; x-tile outer
                for name, xdram, wdram, dst in (
                        ("q", xq_t, wq_t, qT), ("k", xk_t, wk_t, kT)):
                    w_sb = wpool.tile([128, ET, CHD], MM, tag="w",
                                      name=f"w{name}_sb")
                    nc.sync.dma_start(
                        w_sb[:],
                        _mm(wdram).rearrange("(po pi) m -> pi po m", pi=128))
                    pss = [psA.tile([128, QB], F32, tag="ps", name=f"ps_{name}{i}")
                           for i in range(8)]
                    for e in range(ET):
                        x_t = xp.tile([128, S], MM, tag="x", name=f"x_{name}{e}")
                        nc.sync.dma_start(
                            x_t[:], _mm(xdram)[e * 128:(e + 1) * 128, :])
                        for m in range(2):
                            for qb in range(NQB):
                                nc.tensor.matmul(
                                    pss[m * NQB + qb][:],
                                    w_sb[:, e, m * 128:(m + 1) * 128],
                                    x_t[:, qb * QB:(qb + 1) * QB],
                                    start=(e == 0), stop=(e == ET - 1))
                    for m in range(2):
                        for qb in range(NQB):
                            cp = nc.scalar.copy if (m + qb) % 2 else \
                                nc.vector.tensor_copy
                            cp(dst[:, m, qb * QB:(qb + 1) * QB],
                               pss[m * NQB + qb][:])

                # V natural: s-outer; all 8 xv e-tiles stay resident
                # so each s-tile owns one psum accumulation group.
                wv_sb = wpool.tile([128, ET, CHD], MM, tag="w", name="wv_sb")
                nc.sync.dma_start(
                    wv_sb[:],
                    _mm(wv_t).rearrange("(po pi) m -> pi po m", pi=128))
                xv_tiles = []
                for e in range(ET):
                    x_t = xp.tile([128, S], MM, tag="x", name=f"x_v{e}")
                    nc.sync.dma_start(
                        x_t[:], _mm(xv_t)[e * 128:(e + 1) * 128, :])
                    xv_tiles.append(x_t)
                # deferred weight loads ride behind the xv DMAs
                nc.sync.dma_start(
                    wo_sb[:], _mm(wo_t).rearrange("(ct p) n -> p ct n", p=128))
                for s in range(NT):
                    v_ps = psA.tile([128, CHD], F32, tag="ps", name=f"ps_v{s}")
                    for e in range(ET):
                        nc.tensor.matmul(
                            v_ps[:], xv_tiles[e][:, s * 128:(s + 1) * 128],
                            wv_sb[:, e, :],
                            start=(e == 0), stop=(e == ET - 1))
                    src = v_ps[:].rearrange("p (h d) -> p h d", d=HD)
                    dst = v_sb[:, s, :].rearrange("p (h d) -> p h d",
                                                  d=HD + 1)[:, :, 0:HD]
                    cp = nc.scalar.copy if s % 2 else nc.vector.tensor_copy
                    cp(dst, src)

            # ---- phase B: attention, head-PAIRS packed on PE ----
            # Heads 2mh (rows 0-63) and 2mh+1 (rows 64-127) issue scores
            # matmuls into different PE row-groups + different psum banks, so
            # they run concurrently. q is split in halves so both heads'
            # U accumulators fit PSUM ([65, 1024] = 2 banks each).
            u_list = [None] * HG
            with tc.tile_pool(name="psS", bufs=1, space="PSUM") as psS, \
                 tc.tile_pool(name="psU", bufs=1, space="PSUM") as psU:
                # PE warm-up: dense dummy matmuls so the HAM clock gate sits
                # at K=8/8 entering the latency-sensitive B phase.
                trash = psS.tile([128, S // 2], F32, tag="spsA", name="warm")
                for w in range(24):
                    nc.tensor.matmul(
                        trash[:, (w % 2) * QB:(w % 2 + 1) * QB],
                        v_sb[:, 0, 0:128],
                        v_sb[:, 0:2, 0:256],
                        start=True, stop=True)
                SH = S // 2
                for mh in range(2):
                    hA, hB = 2 * mh, 2 * mh + 1
                    uA = usb.tile([HD + 1, S], F32, tag="u", name=f"u{hA}")
                    uB = usb.tile([HD + 1, S], F32, tag="u", name=f"u{hB}")
                    u_list[hA], u_list[hB] = uA, uB
                    for qh in range(2):
                        qo = qh * SH
                        uaccA = psU.tile([HD + 1, SH], F32, tag="uaccA",
                                         name=f"uaccA{mh}_{qh}")
                        uaccB = psU.tile([HD + 1, SH], F32, tag="uaccB",
                                         name=f"uaccB{mh}_{qh}")
                        for t in range(NT):
                            spA = psS.tile([128, SH], F32, tag="spsA",
                                           name=f"spsA{mh}{qh}{t}")
                            spB = psS.tile([128, SH], F32, tag="spsB",
                                           name=f"spsB{mh}{qh}{t}")
                            for j in range(2):
                                for bp, sp in ((0, spA), (64, spB)):
                                    nc.tensor.matmul(
                                        sp[:, j * QB:(j + 1) * QB],
                                        kT[bp:bp + HD, mh,
                                           t * 128:(t + 1) * 128],
                                        qT[bp:bp + HD, mh,
                                           qo + j * QB:qo + (j + 1) * QB],
                                        start=True, stop=True)
                            pA = ptp.tile([128, SH], MM, tag="ptA",
                                          name=f"ptA{mh}{qh}{t}")
                            nc.scalar.activation(pA[:], spA[:], EXP, scale=0.125)
                            pB = ptp.tile([128, SH], MM, tag="ptB",
                                          name=f"ptB{mh}{qh}{t}")
                            kind = B_SCHED[t]
                            if kind == "act":
                                nc.scalar.activation(pB[:], spB[:], EXP,
                                                     scale=0.125)
                            else:
                                sc = escp.tile([128, SH], F32, tag="esc",
                                               name=f"esc{mh}{qh}{t}")
                                nc.vector._custom_dve(
                                    EXP_POLY4, out=sc[:], in0=spB[:],
                                    in1=b1c[:], s0=EXP_B4, s1=EXP_B3,
                                    imm2=EXP_B2)
                                if kind == "dve":
                                    nc.vector._custom_dve(
                                        POW8, out=pB[:], in0=sc[:])
                                else:
                                    s2 = psqp.tile([128, SH], F32, tag="sq2",
                                                   name=f"sq2_{mh}{qh}{t}")
                                    s4 = psqp.tile([128, SH], F32, tag="sq4",
                                                   name=f"sq4_{mh}{qh}{t}")
                                    nc.gpsimd.tensor_tensor(
                                        s2[:], sc[:], sc[:], MULT)
                                    nc.gpsimd.tensor_tensor(
                                        s4[:], s2[:], s2[:], MULT)
                                    nc.gpsimd.tensor_tensor(
                                        pB[:], s4[:], s4[:], MULT)
                            for h2, uacc, p_t in ((hA, uaccA, pA),
                                                  (hB, uaccB, pB)):
                                for j in range(2):
                                    nc.tensor.matmul(
                                        uacc[:, j * QB:(j + 1) * QB],
                                        v_sb[:, t,
                                             h2 * (HD + 1):(h2 + 1) * (HD + 1)],
                                        p_t[:, j * QB:(j + 1) * QB],
                                        start=(t == 0), stop=(t == NT - 1))
                        nc.scalar.copy(uA[:, qo:qo + SH], uaccA[:])
                        nc.scalar.copy(uB[:, qo:qo + SH], uaccB[:])
                    # softmax normalization for this pair: r = 1/sums, then
                    # broadcast r across 64 partitions via a DRAM-bounce DMA
                    # (stride-0 partition reads are legal from DRAM). O^T
                    # overwrites qT. Pair 0's work overlaps pair 1's B loop.
                    # gather the two sums rows to partition base 0 (custom
                    # DVE recip misbehaves on base-64 reads), one recip for
                    # the pair, then DRAM-bounce broadcast per head.
                    rp2 = rpool.tile([2, S], F32, tag="rh", name=f"rp2_{mh}")
                    nc.sync.dma_start(rp2[0:1, :], uA[HD:HD + 1, :])
                    nc.sync.dma_start(rp2[1:2, :], uB[HD:HD + 1, :])
                    r2 = rpool.tile([2, S], F32, tag="rh2", name=f"r2_{mh}")
                    nc.vector.reciprocal_approx_fast(out=r2[:], in_=rp2[:])
                    for h2, u_h in ((hA, uA), (hB, uB)):
                        bp2 = 64 * (h2 % 2)
                        rd = rdram.tile([1, S], F32, name=f"rd{h2}")
                        nc.sync.dma_start(rd[:], r2[h2 % 2:h2 % 2 + 1, :])
                        rb = rpool.tile([HD, S], F32, tag="rb", name=f"rb{h2}",
                                        bufs=3)
                        nc.sync.dma_start(rb[:], rd[:].to_broadcast([HD, S]))
                        for qh2 in range(2):
                            o2 = qh2 * SH
                            eng = nc.gpsimd if (h2 + qh2) % 2 else nc.vector
                            eng.tensor_tensor(
                                qT[bp2:bp2 + HD, mh, o2:o2 + SH],
                                u_h[0:HD, o2:o2 + SH],
                                rb[:, o2:o2 + SH], MULT)

            # ---- phase D: output projection (qT now holds O^T) ----
            with tc.tile_pool(name="psY", bufs=4, space="PSUM") as psY:
                # keep PE busy across the tail of the normalization chain
                tr1 = psY.tile([128, EMB], F32, tag="yps", name="warm1ps")
                for w in range(24):
                    nc.tensor.matmul(tr1[:, 0:QB], warm0[:, 0:128], warm0[:],
                                     start=True, stop=True)
                for s in range(NT):
                    y_ps = psY.tile([128, EMB], F32, tag="yps", name=f"yps{s}")
                    for nb in range(2):
                        for ct in range(2):
                            nc.tensor.matmul(
                                y_ps[:, nb * QB:(nb + 1) * QB],
                                qT[:, ct, s * 128:(s + 1) * 128],
                                wo_sb[:, ct, nb * QB:(nb + 1) * QB],
                                start=(ct == 0), stop=(ct == 1))
                    y_sb = ypool.tile([128, EMB], F32, tag="ysb",
                                      name=f"ysb{s}")
                    cp = nc.scalar.copy if s % 2 else nc.vector.tensor_copy
                    cp(y_sb[:], y_ps[:])
                    nc.sync.dma_start(y[s * 128:(s + 1) * 128, :], y_sb[:])

    nc.compile()
    return nc


def get_nc():
    global _NC
    if _NC is None:
        _NC = _build()
    return _NC


def make_in_maps(query, key, value, Wq, Wk, Wv, Wo):
    import ml_dtypes
    np_dt = {F32R: np.float32, BF16: ml_dtypes.bfloat16,
             FP16: np.float16}[MM]
    query = np.asarray(query, dtype=np.float32)
    key = np.asarray(key, dtype=np.float32)
    value = np.asarray(value, dtype=np.float32)
    Wq = np.asarray(Wq, dtype=np.float32)
    Wk = np.asarray(Wk, dtype=np.float32)
    Wv = np.asarray(Wv, dtype=np.float32)
    Wo = np.asarray(Wo, dtype=np.float32)
    xt = {(n, b): np.ascontiguousarray(x[b].T).astype(np_dt)
          for n, x in (("q", query), ("k", key), ("v", value))
          for b in range(B)}
    in_maps = []
    for c in range(8):
        b, g = divmod(c, 4)
        hs = slice(g * CHD, (g + 1) * CHD)
        in_maps.append({
            "xq_t": xt[("q", b)],
            "xk_t": xt[("k", b)],
            "xv_t": xt[("v", b)],
            "wq_t": np.ascontiguousarray(Wq[hs, :].T).astype(np_dt),
            "wk_t": np.ascontiguousarray(Wk[hs, :].T).astype(np_dt),
            "wv_t": np.ascontiguousarray(Wv[hs, :].T).astype(np_dt),
            "wo_t": np.ascontiguousarray(Wo[:, hs].T).astype(np_dt),
        })
    return in_maps


def gather(results):
    out = np.zeros((B, S, EMB), dtype=np.float32)
    for c in range(8):
        out[c // 4] += results[c]["y"]
    return out


def kernel(**inputs) -> np.ndarray:
    nc = get_nc()
    in_maps = make_in_maps(**inputs)
    res = run_bass_kernel_spmd(nc, in_maps, core_ids=list(range(8)))
    return gather(res.results)


# revision 13
# speedup vs baseline: 1.4637x; 1.4637x over previous
"""Multi-head attention (B=2, S=2048, EMB=1024, H=16, hd=64) on 8 TRN2 cores.

Sharding: core c -> batch b = c//4, head-group g = c%4 (4 heads, 256 emb dims).
Per core (all matmuls bf16: full-rate 1 cyc/row streaming):
  A) Q^T = Wq_g @ x_b^T   [256, 2048]   (transposed layout, contraction on emb)
     K^T = Wk_g @ x_b^T   [256, 2048]
     V   = x_b @ Wv_g^T   [2048, 256]   (natural layout, +ones column per head)
  B) per head h: S^T[k,q] = K_h @ Q_h^T (16 k-tiles x [128, 2048] psum)
     P^T = exp(S^T/8): split between ACT (native Exp) and DVE (custom 2-op
     chain: deg-4 poly p~exp(s/64) then p^8) so the softmax isn't ACT-bound;
     U_aug[65, 2048] += [V_h|1].T @ P^T  (row 64 = softmax sums)
  C) r = 1/sums (DVE recip approx); broadcast r over 64 partitions via
     indicator matmul; O^T = U^T * r on GpSimd (written over the Q^T buffer)
  D) y = O @ Wo_g^T partial [2048, 1024]; host sums the 4 head-group partials.
"""
import numpy as np

import concourse.bass as bass
import concourse.tile as tile
from concourse import bacc, mybir
from concourse.bass_utils import run_bass_kernel_spmd

import os

F32 = mybir.dt.float32
F32R = mybir.dt.float32r
BF16 = mybir.dt.bfloat16
FP16 = mybir.dt.float16
# matmul dtype: bf16 (1 cyc/row, ~6e-3) | fp16 (2 cyc/row, ~1e-3) | f32r
MM_DT_NAME = os.environ.get("MM_DT", "bf16")
MM = {"f32r": F32R, "bf16": BF16, "fp16": FP16}[MM_DT_NAME]
IN_DT = {"f32r": F32, "bf16": BF16, "fp16": FP16}[MM_DT_NAME]
EXP = mybir.ActivationFunctionType.Exp
MULT = mybir.AluOpType.mult

EMB = 1024
S = 2048
B = 2
HG = 4           # heads per core
HD = 64
CHD = HG * HD    # 256 emb dims per core
ET = EMB // 128  # 8 e-tiles
NT = S // 128    # 16 s/k-tiles
QB = 512
NQB = S // QB    # 4

_NC = None

# ---- custom DVE exp: p(s) ~ exp(s/64) (deg-4, a0=1), then p^8 -------------
# minimax-with-a0=1 coefficients for exp(u) on u in [-0.8, 0.8], folded with
# the 1/64 argument scale (b_i = a_i / 64^i).
_A = (0.99930331, 0.49979974, 0.17207327, 0.04243063)
EXP_B1 = _A[0] / 64.0
EXP_B2 = _A[1] / 64.0 ** 2
EXP_B3 = _A[2] / 64.0 ** 3
EXP_B4 = _A[3] / 64.0 ** 4

# exp-tile engine assignment: B-stream odd t -> DVE (poly+pow8), else ACT;
# A-stream on ACT except A_DVE_T. Balanced so ACT ~ DVE per (mh, qh).
B_DVE_T = frozenset((1, 3, 5, 7, 9, 11, 13, 15))
A_DVE_T = frozenset((14,))


def _register_dve_op(name, spec, subdim=False):
    import concourse.dve_ops as dvo
    from concourse.dve_uop import DveOpSpec
    from concourse.dve_spec import lower
    from concourse.dve_spec import _has_src1 as has_src1

    for op in dvo.OPS:
        if op.name == name:
            return op
    opcode = dvo._CUSTOM_DVE_ROW_BASE + len(dvo.OPS)
    assert opcode < 0x20
    dvo._SUB_OPCODE_FOR_NAME[name] = opcode
    shas = {}
    for ver in ("v3", "v4"):
        tmp = DveOpSpec(name=name, opcode=opcode, uops=lower(spec, ver=ver),
                        rd1_en=has_src1(spec))
        shas[ver] = tmp.sha(ver)
    op = dvo.DveOp(name, spec, subdim=subdim, uops_sha=shas)
    dvo.OPS.append(op)
    dvo.CUSTOM_DVE_SPECS[name] = spec
    return op


def _make_exp_ops():
    from concourse.dve_spec import (
        Spec, Src0, C0, C1, C2, C3, One, sq, _spill_c3_to_src1,
    )

    u = Src0
    body = _spill_c3_to_src1(((((u * C0 + C1) * u + C2) * u + C3) * u) + One)

    def _ref_poly(in0, in1, s0, s1, imm2):
        return ((((in0 * s0 + s1) * in0 + imm2) * in0 + in1) * in0
                + np.float32(1.0)).astype(np.float32)

    poly = _register_dve_op("EXP_POLY4_ANT", Spec(body=body, reference=_ref_poly))

    def _ref_pow8(in0, in1, s0, s1, imm2):
        q = (in0 * in0).astype(np.float32)
        q = (q * q).astype(np.float32)
        return (q * q).astype(np.float32)

    pow8 = _register_dve_op("POW8_ANT",
                            Spec(body=sq(sq(sq(Src0))), reference=_ref_pow8))
    return poly, pow8


EXP_POLY4, POW8 = _make_exp_ops()


def _mm(ap):
    """View a dram input AP with the matmul dtype (bitcast only for f32r)."""
    return ap.bitcast(F32R) if MM == F32R else ap


def _build():
    nc = bacc.Bacc("TRN2", target_bir_lowering=False, debug=False)
    xq_t = nc.dram_tensor("xq_t", [EMB, S], IN_DT, kind="ExternalInput").ap()
    xk_t = nc.dram_tensor("xk_t", [EMB, S], IN_DT, kind="ExternalInput").ap()
    xv_t = nc.dram_tensor("xv_t", [EMB, S], IN_DT, kind="ExternalInput").ap()
    wq_t = nc.dram_tensor("wq_t", [EMB, CHD], IN_DT, kind="ExternalInput").ap()
    wk_t = nc.dram_tensor("wk_t", [EMB, CHD], IN_DT, kind="ExternalInput").ap()
    wv_t = nc.dram_tensor("wv_t", [EMB, CHD], IN_DT, kind="ExternalInput").ap()
    wo_t = nc.dram_tensor("wo_t", [CHD, EMB], IN_DT, kind="ExternalInput").ap()
    y = nc.dram_tensor("y", [S, EMB], F32, kind="ExternalOutput").ap()

    with tile.TileContext(nc) as tc:
        with tc.tile_pool(name="const", bufs=1) as cpool, \
             tc.tile_pool(name="wqk", bufs=2) as wpool, \
             tc.tile_pool(name="big", bufs=1) as big, \
             tc.tile_pool(name="usb", bufs=4) as usb, \
             tc.tile_pool(name="xp", bufs=8) as xp, \
             tc.tile_pool(name="pt", bufs=4) as ptp, \
             tc.tile_pool(name="esc", bufs=3) as escp, \
             tc.tile_pool(name="yp", bufs=2) as ypool, \
             tc.tile_pool(name="rp", bufs=2) as rpool, \
             tc.tile_pool(name="rd", bufs=4, space="DRAM") as rdram:

            # ---- static weights (wo DMA deferred past phase A) ----
            wo_sb = cpool.tile([128, 2, EMB], MM, name="wo_sb")
            b1c = cpool.tile([128, 1], F32, name="b1c")
            nc.vector.memset(b1c[:], EXP_B1)

            qT = big.tile([128, 2, S], MM, name="qT")     # later reused as O^T
            kT = big.tile([128, 2, S], MM, name="kT")
            v_sb = big.tile([128, NT, HG * (HD + 1)], MM, name="v_sb")
            if MM == F32R:
                nc.vector.memset(v_sb[:].bitcast(F32), 1.0)
            else:
                nc.vector.memset(v_sb[:], 1.0)     # ones cols survive

            # ---- phase A: projections ----
            warm0 = cpool.tile([128, QB], MM, name="warm0")
            nc.vector.memset(warm0[:], 1.0)
            with tc.tile_pool(name="psA", bufs=8, space="PSUM") as psA:
                # Q^T and K^T: out[m, q] accumulated over e; x-tile outer
                for name, xdram, wdram, dst in (
                        ("q", xq_t, wq_t, qT), ("k", xk_t, wk_t, kT)):
                    w_sb = wpool.tile([128, ET, CHD], MM, tag="w",
                                      name=f"w{name}_sb")
                    nc.sync.dma_start(
                        w_sb[:],
                        _mm(wdram).rearrange("(po pi) m -> pi po m", pi=128))
                    pss = [psA.tile([128, QB], F32, tag="ps", name=f"ps_{name}{i}")
                           for i in range(8)]
                    for e in range(ET):
                        x_t = xp.tile([128, S], MM, tag="x", name=f"x_{name}{e}")
                        nc.sync.dma_start(
                            x_t[:], _mm(xdram)[e * 128:(e + 1) * 128, :])
                        for m in range(2):
                            for qb in range(NQB):
                                nc.tensor.matmul(
                                    pss[m * NQB + qb][:],
                                    w_sb[:, e, m * 128:(m + 1) * 128],
                                    x_t[:, qb * QB:(qb + 1) * QB],
                                    start=(e == 0), stop=(e == ET - 1))
                    for m in range(2):
                        for qb in range(NQB):
                            cp = nc.scalar.copy if (m + qb) % 2 else \
                                nc.vector.tensor_copy
                            cp(dst[:, m, qb * QB:(qb + 1) * QB],
                               pss[m * NQB + qb][:])

                # V natural: s-outer; all 8 xv e-tiles stay resident
                # so each s-tile owns one psum accumulation group.
                wv_sb = wpool.tile([128, ET, CHD], MM, tag="w", name="wv_sb")
                nc.sync.dma_start(
                    wv_sb[:],
                    _mm(wv_t).rearrange("(po pi) m -> pi po m", pi=128))
                xv_tiles = []
                for e in range(ET):
                    x_t = xp.tile([128, S], MM, tag="x", name=f"x_v{e}")
                    nc.sync.dma_start(
                        x_t[:], _mm(xv_t)[e * 128:(e + 1) * 128, :])
                    xv_tiles.append(x_t)
                # deferred weight loads ride behind the xv DMAs
                nc.sync.dma_start(
                    wo_sb[:], _mm(wo_t).rearrange("(ct p) n -> p ct n", p=128))
                for s in range(NT):
                    v_ps = psA.tile([128, CHD], F32, tag="ps", name=f"ps_v{s}")
                    for e in range(ET):
                        nc.tensor.matmul(
                            v_ps[:], xv_tiles[e][:, s * 128:(s + 1) * 128],
                            wv_sb[:, e, :],
                            start=(e == 0), stop=(e == ET - 1))
                    src = v_ps[:].rearrange("p (h d) -> p h d", d=HD)
                    dst = v_sb[:, s, :].rearrange("p (h d) -> p h d",
                                                  d=HD + 1)[:, :, 0:HD]
                    cp = nc.scalar.copy if s % 2 else nc.vector.tensor_copy
                    cp(dst, src)

            # ---- phase B: attention, head-PAIRS packed on PE ----
            # Heads 2mh (rows 0-63) and 2mh+1 (rows 64-127) issue scores
            # matmuls into different PE row-groups + different psum banks, so
            # they run concurrently. q is split in halves so both heads'
            # U accumulators fit PSUM ([65, 1024] = 2 banks each).
            u_list = [None] * HG
            with tc.tile_pool(name="psS", bufs=1, space="PSUM") as psS, \
                 tc.tile_pool(name="psU", bufs=1, space="PSUM") as psU:
                # PE warm-up: dense dummy matmuls so the HAM clock gate sits
                # at K=8/8 entering the latency-sensitive B phase.
                trash = psS.tile([128, S // 2], F32, tag="spsA", name="warm")
                for w in range(24):
                    nc.tensor.matmul(
                        trash[:, (w % 2) * QB:(w % 2 + 1) * QB],
                        v_sb[:, 0, 0:128],
                        v_sb[:, 0:2, 0:256],
                        start=True, stop=True)
                SH = S // 2
                for mh in range(2):
                    hA, hB = 2 * mh, 2 * mh + 1
                    uA = usb.tile([HD + 1, S], F32, tag="u", name=f"u{hA}")
                    uB = usb.tile([HD + 1, S], F32, tag="u", name=f"u{hB}")
                    u_list[hA], u_list[hB] = uA, uB
                    for qh in range(2):
                        qo = qh * SH
                        uaccA = psU.tile([HD + 1, SH], F32, tag="uaccA",
                                         name=f"uaccA{mh}_{qh}")
                        uaccB = psU.tile([HD + 1, SH], F32, tag="uaccB",
                                         name=f"uaccB{mh}_{qh}")

                        def pv(t, p_pair):
                            # PV matmuls for tile t (emitted PV_LAG iters
                            # after its exp so the DVE chain latency never
                            # blocks the in-order PE queue)
                            for h2, uacc, p_t in ((hA, uaccA, p_pair[0]),
                                                  (hB, uaccB, p_pair[1])):
                                for j in range(2):
                                    nc.tensor.matmul(
                                        uacc[:, j * QB:(j + 1) * QB],
                                        v_sb[:, t,
                                             h2 * (HD + 1):(h2 + 1) * (HD + 1)],
                                        p_t[:, j * QB:(j + 1) * QB],
                                        start=(t == 0), stop=(t == NT - 1))

                        PV_LAG = 2
                        pq = []
                        for t in range(NT):
                            spA = psS.tile([128, SH], F32, tag="spsA",
                                           name=f"spsA{mh}{qh}{t}")
                            spB = psS.tile([128, SH], F32, tag="spsB",
                                           name=f"spsB{mh}{qh}{t}")
                            for j in range(2):
                                for bp, sp in ((0, spA), (64, spB)):
                                    nc.tensor.matmul(
                                        sp[:, j * QB:(j + 1) * QB],
                                        kT[bp:bp + HD, mh,
                                           t * 128:(t + 1) * 128],
                                        qT[bp:bp + HD, mh,
                                           qo + j * QB:qo + (j + 1) * QB],
                                        start=True, stop=True)
                            pair = []
                            for st, sp, dve in (("A", spA, t in A_DVE_T),
                                                ("B", spB, t in B_DVE_T)):
                                p_t = ptp.tile([128, SH], MM, tag=f"pt{st}",
                                               name=f"pt{st}{mh}{qh}{t}")
                                if dve:
                                    sc = escp.tile([128, SH], F32, tag="esc",
                                                   name=f"esc{st}{mh}{qh}{t}")
                                    nc.vector._custom_dve(
                                        EXP_POLY4, out=sc[:], in0=sp[:],
                                        in1=b1c[:], s0=EXP_B4, s1=EXP_B3,
                                        imm2=EXP_B2)
                                    nc.vector._custom_dve(
                                        POW8, out=p_t[:], in0=sc[:])
                                else:
                                    nc.scalar.activation(p_t[:], sp[:], EXP,
                                                         scale=0.125)
                                pair.append(p_t)
                            pq.append(pair)
                            if t >= PV_LAG:
                                pv(t - PV_LAG, pq[t - PV_LAG])
                        for t in range(NT - PV_LAG, NT):
                            pv(t, pq[t])
                        nc.scalar.copy(uA[:, qo:qo + SH], uaccA[:])
                        nc.vector.tensor_copy(uB[:, qo:qo + SH], uaccB[:])
                    # softmax normalization for this pair: r = 1/sums, then
                    # broadcast r across 64 partitions via a DRAM-bounce DMA
                    # (stride-0 partition reads are legal from DRAM). O^T
                    # overwrites qT. Pair 0's work overlaps pair 1's B loop.
                    # gather the two sums rows to partition base 0 (custom
                    # DVE recip misbehaves on base-64 reads), one recip for
                    # the pair, then DRAM-bounce broadcast per head.
                    rp2 = rpool.tile([2, S], F32, tag="rh", name=f"rp2_{mh}")
                    nc.sync.dma_start(rp2[0:1, :], uA[HD:HD + 1, :])
                    nc.sync.dma_start(rp2[1:2, :], uB[HD:HD + 1, :])
                    r2 = rpool.tile([2, S], F32, tag="rh2", name=f"r2_{mh}")
                    nc.vector.reciprocal_approx_fast(out=r2[:], in_=rp2[:])
                    for h2, u_h in ((hA, uA), (hB, uB)):
                        bp2 = 64 * (h2 % 2)
                        rd = rdram.tile([1, S], F32, name=f"rd{h2}")
                        nc.sync.dma_start(rd[:], r2[h2 % 2:h2 % 2 + 1, :])
                        rb = rpool.tile([HD, S], F32, tag="rb", name=f"rb{h2}",
                                        bufs=3)
                        nc.sync.dma_start(rb[:], rd[:].to_broadcast([HD, S]))
                        for qh2 in range(2):
                            o2 = qh2 * SH
                            eng = nc.gpsimd if (h2 + qh2) % 2 else nc.vector
                            eng.tensor_tensor(
                                qT[bp2:bp2 + HD, mh, o2:o2 + SH],
                                u_h[0:HD, o2:o2 + SH],
                                rb[:, o2:o2 + SH], MULT)

            # ---- phase D: output projection (qT now holds O^T) ----
            with tc.tile_pool(name="psY", bufs=4, space="PSUM") as psY:
                # keep PE busy across the tail of the normalization chain
                tr1 = psY.tile([128, EMB], F32, tag="yps", name="warm1ps")
                for w in range(24):
                    nc.tensor.matmul(tr1[:, 0:QB], warm0[:, 0:128], warm0[:],
                                     start=True, stop=True)
                for s in range(NT):
                    y_ps = psY.tile([128, EMB], F32, tag="yps", name=f"yps{s}")
                    for nb in range(2):
                        for ct in range(2):
                            nc.tensor.matmul(
                                y_ps[:, nb * QB:(nb + 1) * QB],
                                qT[:, ct, s * 128:(s + 1) * 128],
                                wo_sb[:, ct, nb * QB:(nb + 1) * QB],
                                start=(ct == 0), stop=(ct == 1))
                    y_sb = ypool.tile([128, EMB], F32, tag="ysb",
                                      name=f"ysb{s}")
                    cp = nc.scalar.copy if s % 2 else nc.vector.tensor_copy
                    cp(y_sb[:], y_ps[:])
                    nc.sync.dma_start(y[s * 128:(s + 1) * 128, :], y_sb[:])

    nc.compile()
    return nc


def get_nc():
    global _NC
    if _NC is None:
        _NC = _build()
    return _NC


def make_in_maps(query, key, value, Wq, Wk, Wv, Wo):
    import ml_dtypes
    np_dt = {F32R: np.float32, BF16: ml_dtypes.bfloat16,
             FP16: np.float16}[MM]
    query = np.asarray(query, dtype=np.float32)
    key = np.asarray(key, dtype=np.float32)
    value = np.asarray(value, dtype=np.float32)
    Wq = np.asarray(Wq, dtype=np.float32)
    Wk = np.asarray(Wk, dtype=np.float32)
    Wv = np.asarray(Wv, dtype=np.float32)
    Wo = np.asarray(Wo, dtype=np.float32)
    xt = {(n, b): np.ascontiguousarray(x[b].T).astype(np_dt)
          for n, x in (("q", query), ("k", key), ("v", value))
          for b in range(B)}
    in_maps = []
    for c in range(8):
        b, g = divmod(c, 4)
        hs = slice(g * CHD, (g + 1) * CHD)
        in_maps.append({
            "xq_t": xt[("q", b)],
            "xk_t": xt[("k", b)],
            "xv_t": xt[("v", b)],
            "wq_t": np.ascontiguousarray(Wq[hs, :].T).astype(np_dt),
            "wk_t": np.ascontiguousarray(Wk[hs, :].T).astype(np_dt),
            "wv_t": np.ascontiguousarray(Wv[hs, :].T).astype(np_dt),
            "wo_t": np.ascontiguousarray(Wo[:, hs].T).astype(np_dt),
        })
    return in_maps


def gather(results):
    out = np.zeros((B, S, EMB), dtype=np.float32)
    for c in range(8):
        out[c // 4] += results[c]["y"]
    return out


def kernel(**inputs) -> np.ndarray:
    nc = get_nc()
    in_maps = make_in_maps(**inputs)
    res = run_bass_kernel_spmd(nc, in_maps, core_ids=list(range(8)))
    return gather(res.results)


# revision 18
# speedup vs baseline: 1.4684x; 1.0032x over previous
"""Multi-head attention (B=2, S=2048, EMB=1024, H=16, hd=64) on 8 TRN2 cores.

Sharding: core c -> batch b = c//4, head-group g = c%4 (4 heads, 256 emb dims).
Per core (all matmuls bf16: full-rate 1 cyc/row streaming):
  A) Q^T = Wq_g @ x_b^T   [256, 2048]   (transposed layout, contraction on emb)
     K^T = Wk_g @ x_b^T   [256, 2048]
     V   = x_b @ Wv_g^T   [2048, 256]   (natural layout, +ones column per head)
  B) per head h: S^T[k,q] = K_h @ Q_h^T (16 k-tiles x [128, 2048] psum)
     P^T = exp(S^T/8): split between ACT (native Exp) and DVE (custom 2-op
     chain: deg-4 poly p~exp(s/64) then p^8) so the softmax isn't ACT-bound;
     U_aug[65, 2048] += [V_h|1].T @ P^T  (row 64 = softmax sums)
  C) r = 1/sums (DVE recip approx); broadcast r over 64 partitions via
     indicator matmul; O^T = U^T * r on GpSimd (written over the Q^T buffer)
  D) y = O @ Wo_g^T partial [2048, 1024]; host sums the 4 head-group partials.
"""
import numpy as np

import concourse.bass as bass
import concourse.tile as tile
from concourse import bacc, mybir
from concourse.bass_utils import run_bass_kernel_spmd

import os

F32 = mybir.dt.float32
F32R = mybir.dt.float32r
BF16 = mybir.dt.bfloat16
FP16 = mybir.dt.float16
# matmul dtype: bf16 (1 cyc/row, ~6e-3) | fp16 (2 cyc/row, ~1e-3) | f32r
MM_DT_NAME = os.environ.get("MM_DT", "bf16")
MM = {"f32r": F32R, "bf16": BF16, "fp16": FP16}[MM_DT_NAME]
IN_DT = {"f32r": F32, "bf16": BF16, "fp16": FP16}[MM_DT_NAME]
EXP = mybir.ActivationFunctionType.Exp
MULT = mybir.AluOpType.mult

EMB = 1024
S = 2048
B = 2
HG = 4           # heads per core
HD = 64
CHD = HG * HD    # 256 emb dims per core
ET = EMB // 128  # 8 e-tiles
NT = S // 128    # 16 s/k-tiles
QB = 512
NQB = S // QB    # 4

_NC = None

# ---- custom DVE exp: p(s) ~ exp(s/64) (deg-4, a0=1), then p^8 -------------
# minimax-with-a0=1 coefficients for exp(u) on u in [-0.8, 0.8], folded with
# the 1/64 argument scale (b_i = a_i / 64^i).
_A = (0.99930331, 0.49979974, 0.17207327, 0.04243063)
EXP_B1 = _A[0] / 64.0
EXP_B2 = _A[1] / 64.0 ** 2
EXP_B3 = _A[2] / 64.0 ** 3
EXP_B4 = _A[3] / 64.0 ** 4

# exp-tile engine assignment: with single-buffered score psum the per-t
# period is scores + exp latency; DVE-saturated t's cycle at ~2.44us vs
# ~2.5us for ACT-pair t's, and DVE must stay under its throughput cap:
# 12 of 16 B-tiles on DVE saturates it. A-stream is always ACT.
B_DVE_T = frozenset(t for t in range(16) if t % 4 != 0)
A_DVE_T = frozenset()


def _register_dve_op(name, spec, subdim=False):
    import concourse.dve_ops as dvo
    from concourse.dve_uop import DveOpSpec
    from concourse.dve_spec import lower
    from concourse.dve_spec import _has_src1 as has_src1

    for op in dvo.OPS:
        if op.name == name:
            return op
    opcode = dvo._CUSTOM_DVE_ROW_BASE + len(dvo.OPS)
    assert opcode < 0x20
    dvo._SUB_OPCODE_FOR_NAME[name] = opcode
    shas = {}
    for ver in ("v3", "v4"):
        tmp = DveOpSpec(name=name, opcode=opcode, uops=lower(spec, ver=ver),
                        rd1_en=has_src1(spec))
        shas[ver] = tmp.sha(ver)
    op = dvo.DveOp(name, spec, subdim=subdim, uops_sha=shas)
    dvo.OPS.append(op)
    dvo.CUSTOM_DVE_SPECS[name] = spec
    return op


def _make_exp_ops():
    from concourse.dve_spec import (
        Spec, Src0, C0, C1, C2, C3, One, sq, _spill_c3_to_src1,
    )

    u = Src0
    body = _spill_c3_to_src1(((((u * C0 + C1) * u + C2) * u + C3) * u) + One)

    def _ref_poly(in0, in1, s0, s1, imm2):
        return ((((in0 * s0 + s1) * in0 + imm2) * in0 + in1) * in0
                + np.float32(1.0)).astype(np.float32)

    poly = _register_dve_op("EXP_POLY4_ANT", Spec(body=body, reference=_ref_poly))

    def _ref_pow8(in0, in1, s0, s1, imm2):
        q = (in0 * in0).astype(np.float32)
        q = (q * q).astype(np.float32)
        return (q * q).astype(np.float32)

    pow8 = _register_dve_op("POW8_ANT",
                            Spec(body=sq(sq(sq(Src0))), reference=_ref_pow8))
    return poly, pow8


EXP_POLY4, POW8 = _make_exp_ops()


def _mm(ap):
    """View a dram input AP with the matmul dtype (bitcast only for f32r)."""
    return ap.bitcast(F32R) if MM == F32R else ap


def _build():
    nc = bacc.Bacc("TRN2", target_bir_lowering=False, debug=False)
    xq_t = nc.dram_tensor("xq_t", [EMB, S], IN_DT, kind="ExternalInput").ap()
    xk_t = nc.dram_tensor("xk_t", [EMB, S], IN_DT, kind="ExternalInput").ap()
    xv_t = nc.dram_tensor("xv_t", [EMB, S], IN_DT, kind="ExternalInput").ap()
    # weights pre-arranged on host to [128, ET*CHD] / [128, 2*EMB] so the
    # load is 128 contiguous 4KB descriptors instead of 1024 strided ones
    wq_t = nc.dram_tensor("wq_t", [128, ET * CHD], IN_DT,
                          kind="ExternalInput").ap()
    wk_t = nc.dram_tensor("wk_t", [128, ET * CHD], IN_DT,
                          kind="ExternalInput").ap()
    wv_t = nc.dram_tensor("wv_t", [128, ET * CHD], IN_DT,
                          kind="ExternalInput").ap()
    wo_t = nc.dram_tensor("wo_t", [128, 2 * EMB], IN_DT,
                          kind="ExternalInput").ap()
    y = nc.dram_tensor("y", [S, EMB], F32, kind="ExternalOutput").ap()

    with tile.TileContext(nc) as tc:
        with tc.tile_pool(name="const", bufs=1) as cpool, \
             tc.tile_pool(name="wqk", bufs=2) as wpool, \
             tc.tile_pool(name="big", bufs=1) as big, \
             tc.tile_pool(name="usb", bufs=4) as usb, \
             tc.tile_pool(name="xp", bufs=8) as xp, \
             tc.tile_pool(name="pt", bufs=4) as ptp, \
             tc.tile_pool(name="esc", bufs=3) as escp, \
             tc.tile_pool(name="yp", bufs=2) as ypool, \
             tc.tile_pool(name="rp", bufs=2) as rpool, \
             tc.tile_pool(name="rd", bufs=4, space="DRAM") as rdram:

            # ---- static weights (wo DMA deferred past phase A) ----
            wo_sb = cpool.tile([128, 2, EMB], MM, name="wo_sb")
            b1c = cpool.tile([128, 1], F32, name="b1c")
            nc.vector.memset(b1c[:], EXP_B1)

            qT = big.tile([128, 2, S], MM, name="qT")     # later reused as O^T
            kT = big.tile([128, 2, S], MM, name="kT")
            v_sb = big.tile([128, NT, HG * (HD + 1)], MM, name="v_sb")
            if MM == F32R:
                nc.vector.memset(v_sb[:].bitcast(F32), 1.0)
            else:
                nc.vector.memset(v_sb[:], 1.0)     # ones cols survive

            # ---- phase A: projections ----
            warm0 = cpool.tile([128, QB], MM, name="warm0")
            nc.vector.memset(warm0[:], 1.0)
            with tc.tile_pool(name="psA", bufs=8, space="PSUM") as psA:
                # Q^T and K^T: out[m, q] accumulated over e; m-halves use
                # 4 psum banks each so one half's copies overlap the other
                # half's matmuls
                for name, xdram, wdram, dst in (
                        ("q", xq_t, wq_t, qT), ("k", xk_t, wk_t, kT)):
                    w_sb = wpool.tile([128, ET, CHD], MM, tag="w",
                                      name=f"w{name}_sb")
                    nc.sync.dma_start(
                        w_sb[:].rearrange("pi po m -> pi (po m)"),
                        _mm(wdram))
                    xts = []
                    for e in range(ET):
                        x_t = xp.tile([128, S], MM, tag="x", name=f"x_{name}{e}")
                        nc.sync.dma_start(
                            x_t[:], _mm(xdram)[e * 128:(e + 1) * 128, :])
                        xts.append(x_t)
                    for m in range(2):
                        pss = [psA.tile([128, QB], F32, tag="ps",
                                        name=f"ps_{name}{m}{i}")
                               for i in range(NQB)]
                        for e in range(ET):
                            for qb in range(NQB):
                                nc.tensor.matmul(
                                    pss[qb][:],
                                    w_sb[:, e, m * 128:(m + 1) * 128],
                                    xts[e][:, qb * QB:(qb + 1) * QB],
                                    start=(e == 0), stop=(e == ET - 1))
                        for qb in range(NQB):
                            cp = nc.scalar.copy if qb % 2 else \
                                nc.vector.tensor_copy
                            cp(dst[:, m, qb * QB:(qb + 1) * QB],
                               pss[qb][:])

                # V natural: s-outer; all 8 xv e-tiles stay resident
                # so each s-tile owns one psum accumulation group.
                wv_sb = wpool.tile([128, ET, CHD], MM, tag="w", name="wv_sb")
                nc.sync.dma_start(
                    wv_sb[:].rearrange("pi po m -> pi (po m)"),
                    _mm(wv_t))
                xv_tiles = []
                for e in range(ET):
                    x_t = xp.tile([128, S], MM, tag="x", name=f"x_v{e}")
                    nc.sync.dma_start(
                        x_t[:], _mm(xv_t)[e * 128:(e + 1) * 128, :])
                    xv_tiles.append(x_t)
                # deferred weight loads ride behind the xv DMAs
                nc.sync.dma_start(
                    wo_sb[:], _mm(wo_t).rearrange("(ct p) n -> p ct n", p=128))
                for s in range(NT):
                    v_ps = psA.tile([128, CHD], F32, tag="ps", name=f"ps_v{s}")
                    for e in range(ET):
                        nc.tensor.matmul(
                            v_ps[:], xv_tiles[e][:, s * 128:(s + 1) * 128],
                            wv_sb[:, e, :],
                            start=(e == 0), stop=(e == ET - 1))
                    src = v_ps[:].rearrange("p (h d) -> p h d", d=HD)
                    dst = v_sb[:, s, :].rearrange("p (h d) -> p h d",
                                                  d=HD + 1)[:, :, 0:HD]
                    cp = nc.scalar.copy if s % 2 else nc.vector.tensor_copy
                    cp(dst, src)

            # ---- phase B: attention, head-PAIRS packed on PE ----
            # Heads 2mh (rows 0-63) and 2mh+1 (rows 64-127) issue scores
            # matmuls into different PE row-groups + different psum banks, so
            # they run concurrently. q is split in halves so both heads'
            # U accumulators fit PSUM ([65, 1024] = 2 banks each).
            u_list = [None] * HG
            with tc.tile_pool(name="psS", bufs=1, space="PSUM") as psS, \
                 tc.tile_pool(name="psU", bufs=1, space="PSUM") as psU:
                # PE warm-up: dense dummy matmuls so the HAM clock gate sits
                # at K=8/8 entering the latency-sensitive B phase.
                trash = psS.tile([128, S // 2], F32, tag="spsA", name="warm")
                for w in range(24):
                    nc.tensor.matmul(
                        trash[:, (w % 2) * QB:(w % 2 + 1) * QB],
                        v_sb[:, 0, 0:128],
                        v_sb[:, 0:2, 0:256],
                        start=True, stop=True)
                SH = S // 2
                for mh in range(2):
                    hA, hB = 2 * mh, 2 * mh + 1
                    uA = usb.tile([HD + 1, S], F32, tag="u", name=f"u{hA}")
                    uB = usb.tile([HD + 1, S], F32, tag="u", name=f"u{hB}")
                    u_list[hA], u_list[hB] = uA, uB
                    for qh in range(2):
                        qo = qh * SH
                        uaccA = psU.tile([HD + 1, SH], F32, tag="uaccA",
                                         name=f"uaccA{mh}_{qh}")
                        uaccB = psU.tile([HD + 1, SH], F32, tag="uaccB",
                                         name=f"uaccB{mh}_{qh}")

                        def pv(t, p_pair):
                            # PV matmuls for tile t (emitted PV_LAG iters
                            # after its exp so the DVE chain latency never
                            # blocks the in-order PE queue)
                            for h2, uacc, p_t in ((hA, uaccA, p_pair[0]),
                                                  (hB, uaccB, p_pair[1])):
                                for j in range(2):
                                    nc.tensor.matmul(
                                        uacc[:, j * QB:(j + 1) * QB],
                                        v_sb[:, t,
                                             h2 * (HD + 1):(h2 + 1) * (HD + 1)],
                                        p_t[:, j * QB:(j + 1) * QB],
                                        start=(t == 0), stop=(t == NT - 1))

                        PV_LAG = 2
                        pq = []
                        for t in range(NT):
                            spA = psS.tile([128, SH], F32, tag="spsA",
                                           name=f"spsA{mh}{qh}{t}")
                            spB = psS.tile([128, SH], F32, tag="spsB",
                                           name=f"spsB{mh}{qh}{t}")
                            for j in range(2):
                                for bp, sp in ((0, spA), (64, spB)):
                                    nc.tensor.matmul(
                                        sp[:, j * QB:(j + 1) * QB],
                                        kT[bp:bp + HD, mh,
                                           t * 128:(t + 1) * 128],
                                        qT[bp:bp + HD, mh,
                                           qo + j * QB:qo + (j + 1) * QB],
                                        start=True, stop=True)
                            pair = []
                            for st, sp, dve in (("A", spA, t in A_DVE_T),
                                                ("B", spB, t in B_DVE_T)):
                                p_t = ptp.tile([128, SH], MM, tag=f"pt{st}",
                                               name=f"pt{st}{mh}{qh}{t}")
                                if dve:
                                    sc = escp.tile([128, SH], F32, tag="esc",
                                                   name=f"esc{st}{mh}{qh}{t}")
                                    nc.vector._custom_dve(
                                        EXP_POLY4, out=sc[:], in0=sp[:],
                                        in1=b1c[:], s0=EXP_B4, s1=EXP_B3,
                                        imm2=EXP_B2)
                                    nc.vector._custom_dve(
                                        POW8, out=p_t[:], in0=sc[:])
                                else:
                                    nc.scalar.activation(p_t[:], sp[:], EXP,
                                                         scale=0.125)
                                pair.append(p_t)
                            pq.append(pair)
                            if t >= PV_LAG:
                                pv(t - PV_LAG, pq[t - PV_LAG])
                        for t in range(NT - PV_LAG, NT):
                            pv(t, pq[t])
                        nc.scalar.copy(uA[:, qo:qo + SH], uaccA[:])
                        nc.vector.tensor_copy(uB[:, qo:qo + SH], uaccB[:])
                        # softmax normalization for this (pair, q-half):
                        # r = 1/sums (row 64), broadcast r across the 64
                        # partitions via a DRAM-bounce DMA (stride-0
                        # partition reads are legal from DRAM), O^T = U * r
                        # overwrites qT. All halves except the last overlap
                        # later B work (Pool); the last exposed half goes to
                        # DVE which is free by then. Doing this per q-half
                        # lets phase D's first s-tiles start right at B end,
                        # which also keeps the PE HAM-warm through the tail.
                        rp2 = rpool.tile([2, SH], F32, tag="rh",
                                         name=f"rp2_{mh}{qh}")
                        nc.sync.dma_start(rp2[0:1, :],
                                          uA[HD:HD + 1, qo:qo + SH])
                        nc.sync.dma_start(rp2[1:2, :],
                                          uB[HD:HD + 1, qo:qo + SH])
                        r2 = rpool.tile([2, SH], F32, tag="rh2",
                                        name=f"r2_{mh}{qh}")
                        nc.vector.reciprocal_approx_fast(out=r2[:], in_=rp2[:])
                        last = (mh == 1 and qh == 1)
                        for h2, u_h in ((hA, uA), (hB, uB)):
                            bp2 = 64 * (h2 % 2)
                            rd = rdram.tile([1, SH], F32, name=f"rd{h2}_{qh}")
                            nc.sync.dma_start(rd[:],
                                              r2[h2 % 2:h2 % 2 + 1, :])
                            rb = rpool.tile([HD, SH], F32, tag="rb",
                                            name=f"rb{h2}_{qh}", bufs=3)
                            nc.sync.dma_start(rb[:],
                                              rd[:].to_broadcast([HD, SH]))
                            eng = nc.vector if last else nc.gpsimd
                            eng.tensor_tensor(
                                qT[bp2:bp2 + HD, mh, qo:qo + SH],
                                u_h[0:HD, qo:qo + SH],
                                rb[:], MULT)

            # ---- phase D: output projection (qT now holds O^T) ----
            with tc.tile_pool(name="psY", bufs=4, space="PSUM") as psY:
                # keep PE busy across the tail of the normalization chain
                tr1 = psY.tile([128, EMB], F32, tag="yps", name="warm1ps")
                for w in range(8):
                    nc.tensor.matmul(tr1[:, 0:QB], warm0[:, 0:128], warm0[:],
                                     start=True, stop=True)
                for s in range(NT):
                    y_ps = psY.tile([128, EMB], F32, tag="yps", name=f"yps{s}")
                    for nb in range(2):
                        for ct in range(2):
                            nc.tensor.matmul(
                                y_ps[:, nb * QB:(nb + 1) * QB],
                                qT[:, ct, s * 128:(s + 1) * 128],
                                wo_sb[:, ct, nb * QB:(nb + 1) * QB],
                                start=(ct == 0), stop=(ct == 1))
                    y_sb = ypool.tile([128, EMB], F32, tag="ysb",
                                      name=f"ysb{s}")
                    cp = nc.scalar.copy if s % 2 else nc.vector.tensor_copy
                    cp(y_sb[:], y_ps[:])
                    nc.sync.dma_start(y[s * 128:(s + 1) * 128, :], y_sb[:])

    nc.compile()
    return nc


def get_nc():
    global _NC
    if _NC is None:
        _NC = _build()
    return _NC


def make_in_maps(query, key, value, Wq, Wk, Wv, Wo):
    import ml_dtypes
    np_dt = {F32R: np.float32, BF16: ml_dtypes.bfloat16,
             FP16: np.float16}[MM]
    query = np.asarray(query, dtype=np.float32)
    key = np.asarray(key, dtype=np.float32)
    value = np.asarray(value, dtype=np.float32)
    Wq = np.asarray(Wq, dtype=np.float32)
    Wk = np.asarray(Wk, dtype=np.float32)
    Wv = np.asarray(Wv, dtype=np.float32)
    Wo = np.asarray(Wo, dtype=np.float32)
    xt = {(n, b): np.ascontiguousarray(x[b].T).astype(np_dt)
          for n, x in (("q", query), ("k", key), ("v", value))
          for b in range(B)}
    in_maps = []
    for c in range(8):
        b, g = divmod(c, 4)
        hs = slice(g * CHD, (g + 1) * CHD)
        in_maps.append({
            "xq_t": xt[("q", b)],
            "xk_t": xt[("k", b)],
            "xv_t": xt[("v", b)],
            "wq_t": np.ascontiguousarray(Wq[hs, :].T).astype(np_dt),
            "wk_t": np.ascontiguousarray(Wk[hs, :].T).astype(np_dt),
            "wv_t": np.ascontiguousarray(Wv[hs, :].T).astype(np_dt),
            "wo_t": np.ascontiguousarray(Wo[:, hs].T).astype(np_dt),
        })
    return in_maps


def gather(results):
    out = np.zeros((B, S, EMB), dtype=np.float32)
    for c in range(8):
        out[c // 4] += results[c]["y"]
    return out


def kernel(**inputs) -> np.ndarray:
    nc = get_nc()
    in_maps = make_in_maps(**inputs)
    res = run_bass_kernel_spmd(nc, in_maps, core_ids=list(range(8)))
    return gather(res.results)


# revision 28
# speedup vs baseline: 1.6240x; 1.1059x over previous
"""Multi-head attention (B=2, S=2048, EMB=1024, H=16, hd=64) on 8 TRN2 cores.

Sharding: core c -> batch b = c//4, head-group g = c%4 (4 heads, 256 emb dims).
Per core (all matmuls bf16: full-rate 1 cyc/row streaming):
  A) Q^T = Wq_g @ x_b^T   [256, 2048]   (transposed layout, contraction on emb)
     K^T = Wk_g @ x_b^T   [256, 2048]
     V   = x_b @ Wv_g^T   [2048, 256]   (natural layout, +ones column per head)
  B) per head h: S^T[k,q] = K_h @ Q_h^T (16 k-tiles x [128, 2048] psum)
     P^T = exp(S^T/8): split between ACT (native Exp) and DVE (custom 2-op
     chain: deg-4 poly p~exp(s/64) then p^8) so the softmax isn't ACT-bound;
     U_aug[65, 2048] += [V_h|1].T @ P^T  (row 64 = softmax sums)
  C) r = 1/sums (DVE recip approx); broadcast r over 64 partitions via
     indicator matmul; O^T = U^T * r on GpSimd (written over the Q^T buffer)
  D) y = O @ Wo_g^T partial [2048, 1024]; host sums the 4 head-group partials.
"""
import numpy as np

import concourse.bass as bass
import concourse.tile as tile
from concourse import bacc, mybir
from concourse.bass_utils import run_bass_kernel_spmd

import os

F32 = mybir.dt.float32
F32R = mybir.dt.float32r
BF16 = mybir.dt.bfloat16
FP16 = mybir.dt.float16
# matmul dtype: bf16 (1 cyc/row, ~6e-3) | fp16 (2 cyc/row, ~1e-3) | f32r
MM_DT_NAME = os.environ.get("MM_DT", "bf16")
MM = {"f32r": F32R, "bf16": BF16, "fp16": FP16}[MM_DT_NAME]
IN_DT = {"f32r": F32, "bf16": BF16, "fp16": FP16}[MM_DT_NAME]
EXP = mybir.ActivationFunctionType.Exp
MULT = mybir.AluOpType.mult

EMB = 1024
S = 2048
B = 2
HG = 4           # heads per core
HD = 64
CHD = HG * HD    # 256 emb dims per core
ET = EMB // 128  # 8 e-tiles
NT = S // 128    # 16 s/k-tiles
QB = 512
NQB = S // QB    # 4

_NC = None

# ---- custom DVE exp: p(s) ~ exp(s/64) (deg-4, a0=1), then p^8 -------------
# minimax-with-a0=1 coefficients for exp(u) on u in [-0.8, 0.8], folded with
# the 1/64 argument scale (b_i = a_i / 64^i).
_A = (0.99930331, 0.49979974, 0.17207327, 0.04243063)
EXP_B1 = _A[0] / 64.0
EXP_B2 = _A[1] / 64.0 ** 2
EXP_B3 = _A[2] / 64.0 ** 3
EXP_B4 = _A[3] / 64.0 ** 4

# exp-tile engine assignment: scores psum is split into [128, 512] j-half
# tiles (1 PSUM bank each, bufs=2) so the j-halves double-buffer each other
# and the exp engines never gate the next scores matmul. Balance ACT vs DVE
# by throughput: A-stream on ACT; 10 of 16 B-stream t's on DVE.
B_DVE_T = frozenset((1, 2, 4, 5, 7, 8, 10, 11, 13, 14))
A_DVE_T = frozenset()


def _register_dve_op(name, spec, subdim=False):
    import concourse.dve_ops as dvo
    from concourse.dve_uop import DveOpSpec
    from concourse.dve_spec import lower
    from concourse.dve_spec import _has_src1 as has_src1

    for op in dvo.OPS:
        if op.name == name:
            return op
    opcode = dvo._CUSTOM_DVE_ROW_BASE + len(dvo.OPS)
    assert opcode < 0x20
    dvo._SUB_OPCODE_FOR_NAME[name] = opcode
    shas = {}
    for ver in ("v3", "v4"):
        tmp = DveOpSpec(name=name, opcode=opcode, uops=lower(spec, ver=ver),
                        rd1_en=has_src1(spec))
        shas[ver] = tmp.sha(ver)
    op = dvo.DveOp(name, spec, subdim=subdim, uops_sha=shas)
    dvo.OPS.append(op)
    dvo.CUSTOM_DVE_SPECS[name] = spec
    return op


def _make_exp_ops():
    from concourse.dve_spec import (
        Spec, Src0, C0, C1, C2, C3, One, sq, _spill_c3_to_src1,
    )

    u = Src0
    body = _spill_c3_to_src1(((((u * C0 + C1) * u + C2) * u + C3) * u) + One)

    def _ref_poly(in0, in1, s0, s1, imm2):
        return ((((in0 * s0 + s1) * in0 + imm2) * in0 + in1) * in0
                + np.float32(1.0)).astype(np.float32)

    poly = _register_dve_op("EXP_POLY4_ANT", Spec(body=body, reference=_ref_poly))

    def _ref_pow8(in0, in1, s0, s1, imm2):
        q = (in0 * in0).astype(np.float32)
        q = (q * q).astype(np.float32)
        return (q * q).astype(np.float32)

    pow8 = _register_dve_op("POW8_ANT",
                            Spec(body=sq(sq(sq(Src0))), reference=_ref_pow8))
    return poly, pow8


EXP_POLY4, POW8 = _make_exp_ops()


def _mm(ap):
    """View a dram input AP with the matmul dtype (bitcast only for f32r)."""
    return ap.bitcast(F32R) if MM == F32R else ap


def _build():
    nc = bacc.Bacc("TRN2", target_bir_lowering=False, debug=False)
    xq_t = nc.dram_tensor("xq_t", [EMB, S], IN_DT, kind="ExternalInput").ap()
    xk_t = nc.dram_tensor("xk_t", [EMB, S], IN_DT, kind="ExternalInput").ap()
    xv_t = nc.dram_tensor("xv_t", [EMB, S], IN_DT, kind="ExternalInput").ap()
    # weights pre-arranged on host to [128, ET*CHD] / [128, 2*EMB] so the
    # load is 128 contiguous 4KB descriptors instead of 1024 strided ones
    wq_t = nc.dram_tensor("wq_t", [128, ET * CHD], IN_DT,
                          kind="ExternalInput").ap()
    wk_t = nc.dram_tensor("wk_t", [128, ET * CHD], IN_DT,
                          kind="ExternalInput").ap()
    wv_t = nc.dram_tensor("wv_t", [128, ET * CHD], IN_DT,
                          kind="ExternalInput").ap()
    wo_t = nc.dram_tensor("wo_t", [128, 2 * EMB], IN_DT,
                          kind="ExternalInput").ap()
    # partial outputs leave in the matmul dtype: halves the output-DMA
    # drain at the kernel tail; the host gather accumulates in fp32
    y_dt = F32 if MM == F32R else MM
    y = nc.dram_tensor("y", [S, EMB], y_dt, kind="ExternalOutput").ap()

    with tile.TileContext(nc) as tc:
        with tc.tile_pool(name="const", bufs=1) as cpool, \
             tc.tile_pool(name="wqk", bufs=2) as wpool, \
             tc.tile_pool(name="big", bufs=1) as big, \
             tc.tile_pool(name="usb", bufs=4) as usb, \
             tc.tile_pool(name="xp", bufs=8) as xp, \
             tc.tile_pool(name="pt", bufs=4) as ptp, \
             tc.tile_pool(name="esc", bufs=3) as escp, \
             tc.tile_pool(name="yp", bufs=2) as ypool, \
             tc.tile_pool(name="rp", bufs=2) as rpool, \
             tc.tile_pool(name="rd", bufs=4, space="DRAM") as rdram:

            # ---- static weights (wo DMA deferred past phase A) ----
            wo_sb = cpool.tile([128, 2, EMB], MM, name="wo_sb")
            b1c = cpool.tile([128, 1], F32, name="b1c")
            nc.vector.memset(b1c[:], EXP_B1)

            qT = big.tile([128, 2, S], MM, name="qT")     # later reused as O^T
            kT = big.tile([128, 2, S], MM, name="kT")
            v_sb = big.tile([128, NT, HG * (HD + 1)], MM, name="v_sb")
            if MM == F32R:
                nc.vector.memset(v_sb[:].bitcast(F32), 1.0)
            else:
                nc.vector.memset(v_sb[:], 1.0)     # ones cols survive

            # ---- phase A: projections ----
            warm0 = cpool.tile([128, QB], MM, name="warm0")
            nc.vector.memset(warm0[:], 1.0)
            with tc.tile_pool(name="psA", bufs=8, space="PSUM") as psA:
                # Q^T and K^T: out[m, q] accumulated over e; m-halves use
                # 4 psum banks each so one half's copies overlap the other
                # half's matmuls
                for name, xdram, wdram, dst in (
                        ("q", xq_t, wq_t, qT), ("k", xk_t, wk_t, kT)):
                    w_sb = wpool.tile([128, ET, CHD], MM, tag="w",
                                      name=f"w{name}_sb")
                    nc.sync.dma_start(
                        w_sb[:].rearrange("pi po m -> pi (po m)"),
                        _mm(wdram))
                    xts = []
                    for e in range(ET):
                        x_t = xp.tile([128, S], MM, tag="x", name=f"x_{name}{e}")
                        nc.sync.dma_start(
                            x_t[:], _mm(xdram)[e * 128:(e + 1) * 128, :])
                        xts.append(x_t)
                    for m in range(2):
                        pss = [psA.tile([128, QB], F32, tag="ps",
                                        name=f"ps_{name}{m}{i}")
                               for i in range(NQB)]
                        for e in range(ET):
                            for qb in range(NQB):
                                nc.tensor.matmul(
                                    pss[qb][:],
                                    w_sb[:, e, m * 128:(m + 1) * 128],
                                    xts[e][:, qb * QB:(qb + 1) * QB],
                                    start=(e == 0), stop=(e == ET - 1))
                        for qb in range(NQB):
                            cp = nc.scalar.copy if qb % 2 else \
                                nc.vector.tensor_copy
                            cp(dst[:, m, qb * QB:(qb + 1) * QB],
                               pss[qb][:])

                # V natural: s-outer; all 8 xv e-tiles stay resident
                # so each s-tile owns one psum accumulation group.
                wv_sb = wpool.tile([128, ET, CHD], MM, tag="w", name="wv_sb")
                nc.sync.dma_start(
                    wv_sb[:].rearrange("pi po m -> pi (po m)"),
                    _mm(wv_t))
                xv_tiles = []
                for e in range(ET):
                    x_t = xp.tile([128, S], MM, tag="x", name=f"x_v{e}")
                    nc.sync.dma_start(
                        x_t[:], _mm(xv_t)[e * 128:(e + 1) * 128, :])
                    xv_tiles.append(x_t)
                # deferred weight loads ride behind the xv DMAs
                nc.sync.dma_start(
                    wo_sb[:].rearrange("p ct n -> p (ct n)"), _mm(wo_t))
                for s in range(NT):
                    v_ps = psA.tile([128, CHD], F32, tag="ps", name=f"ps_v{s}")
                    for e in range(ET):
                        nc.tensor.matmul(
                            v_ps[:], xv_tiles[e][:, s * 128:(s + 1) * 128],
                            wv_sb[:, e, :],
                            start=(e == 0), stop=(e == ET - 1))
                    src = v_ps[:].rearrange("p (h d) -> p h d", d=HD)
                    dst = v_sb[:, s, :].rearrange("p (h d) -> p h d",
                                                  d=HD + 1)[:, :, 0:HD]
                    cp = nc.scalar.copy if s % 2 else nc.vector.tensor_copy
                    cp(dst, src)

            # ---- phase B: attention, head-PAIRS packed on PE ----
            # Heads 2mh (rows 0-63) and 2mh+1 (rows 64-127) issue scores
            # matmuls into different PE row-groups + different psum banks, so
            # they run concurrently. q is split in halves so both heads'
            # U accumulators fit PSUM ([65, 1024] = 2 banks each).
            u_list = [None] * HG
            with tc.tile_pool(name="psS", bufs=2, space="PSUM") as psS, \
                 tc.tile_pool(name="psU", bufs=1, space="PSUM") as psU:
                # PE warm-up: dense dummy matmuls so the HAM clock gate sits
                # at K=8/8 entering the latency-sensitive B phase.
                trash = psS.tile([128, QB], F32, tag="spsA", name="warm")
                for w in range(24):
                    nc.tensor.matmul(
                        trash[:],
                        v_sb[:, 0, 0:128],
                        v_sb[:, 0:2, 0:256],
                        start=True, stop=True)
                SH = S // 2
                for mh in range(2):
                    hA, hB = 2 * mh, 2 * mh + 1
                    uA = usb.tile([HD + 1, S], F32, tag="u", name=f"u{hA}")
                    uB = usb.tile([HD + 1, S], F32, tag="u", name=f"u{hB}")
                    u_list[hA], u_list[hB] = uA, uB
                    for qh in range(2):
                        qo = qh * SH
                        uaccA = psU.tile([HD + 1, SH], F32, tag="uaccA",
                                         name=f"uaccA{mh}_{qh}")
                        uaccB = psU.tile([HD + 1, SH], F32, tag="uaccB",
                                         name=f"uaccB{mh}_{qh}")

                        def pv(t, p_pair):
                            # PV matmuls for tile t (emitted PV_LAG iters
                            # after its exp so the DVE chain latency never
                            # blocks the in-order PE queue)
                            for h2, uacc, p_t in ((hA, uaccA, p_pair[0]),
                                                  (hB, uaccB, p_pair[1])):
                                for j in range(2):
                                    nc.tensor.matmul(
                                        uacc[:, j * QB:(j + 1) * QB],
                                        v_sb[:, t,
                                             h2 * (HD + 1):(h2 + 1) * (HD + 1)],
                                        p_t[:, j * QB:(j + 1) * QB],
                                        start=(t == 0), stop=(t == NT - 1))

                        PV_LAG = 2
                        pq = []
                        for t in range(NT):
                            pA = ptp.tile([128, SH], MM, tag="ptA",
                                          name=f"ptA{mh}{qh}{t}")
                            pB = ptp.tile([128, SH], MM, tag="ptB",
                                          name=f"ptB{mh}{qh}{t}")
                            for j in range(2):
                                js = slice(j * QB, (j + 1) * QB)
                                sps = []
                                for bp, st in ((0, "A"), (64, "B")):
                                    sp = psS.tile([128, QB], F32,
                                                  tag=f"sps{st}",
                                                  name=f"sps{st}{mh}{qh}{t}{j}")
                                    nc.tensor.matmul(
                                        sp[:],
                                        kT[bp:bp + HD, mh,
                                           t * 128:(t + 1) * 128],
                                        qT[bp:bp + HD, mh,
                                           qo + j * QB:qo + (j + 1) * QB],
                                        start=True, stop=True)
                                    sps.append(sp)
                                for sp, p_t, dve in (
                                        (sps[0], pA, t in A_DVE_T),
                                        (sps[1], pB, t in B_DVE_T)):
                                    if dve:
                                        sc = escp.tile(
                                            [128, QB], F32, tag="esc",
                                            name=f"esc{mh}{qh}{t}{j}")
                                        nc.vector._custom_dve(
                                            EXP_POLY4, out=sc[:], in0=sp[:],
                                            in1=b1c[:], s0=EXP_B4, s1=EXP_B3,
                                            imm2=EXP_B2)
                                        nc.vector._custom_dve(
                                            POW8, out=p_t[:, js], in0=sc[:])
                                    else:
                                        nc.scalar.activation(
                                            p_t[:, js], sp[:], EXP,
                                            scale=0.125)
                            pq.append((pA, pB))
                            if t >= PV_LAG:
                                pv(t - PV_LAG, pq[t - PV_LAG])
                        for t in range(NT - PV_LAG, NT):
                            pv(t, pq[t])
                        nc.scalar.copy(uA[:, qo:qo + SH], uaccA[:])
                        nc.vector.tensor_copy(uB[:, qo:qo + SH], uaccB[:])
                        # softmax normalization for this (pair, q-half):
                        # r = 1/sums (row 64), broadcast r across the 64
                        # partitions via a DRAM-bounce DMA (stride-0
                        # partition reads are legal from DRAM), O^T = U * r
                        # overwrites qT. All halves except the last overlap
                        # later B work (Pool); the last exposed half goes to
                        # DVE which is free by then. Doing this per q-half
                        # lets phase D's first s-tiles start right at B end,
                        # which also keeps the PE HAM-warm through the tail.
                        rp2 = rpool.tile([2, SH], F32, tag="rh",
                                         name=f"rp2_{mh}{qh}")
                        nc.sync.dma_start(rp2[0:1, :],
                                          uA[HD:HD + 1, qo:qo + SH])
                        nc.sync.dma_start(rp2[1:2, :],
                                          uB[HD:HD + 1, qo:qo + SH])
                        r2 = rpool.tile([2, SH], F32, tag="rh2",
                                        name=f"r2_{mh}{qh}")
                        nc.vector.reciprocal_approx_fast(out=r2[:], in_=rp2[:])
                        last = (mh == 1 and qh == 1)
                        for h2, u_h in ((hA, uA), (hB, uB)):
                            bp2 = 64 * (h2 % 2)
                            rd = rdram.tile([1, SH], F32, name=f"rd{h2}_{qh}")
                            nc.sync.dma_start(rd[:],
                                              r2[h2 % 2:h2 % 2 + 1, :])
                            rb = rpool.tile([HD, SH], F32, tag="rb",
                                            name=f"rb{h2}_{qh}", bufs=3)
                            nc.sync.dma_start(rb[:],
                                              rd[:].to_broadcast([HD, SH]))
                            eng = nc.vector if last else nc.gpsimd
                            eng.tensor_tensor(
                                qT[bp2:bp2 + HD, mh, qo:qo + SH],
                                u_h[0:HD, qo:qo + SH],
                                rb[:], MULT)

            # ---- phase D: output projection (qT now holds O^T) ----
            with tc.tile_pool(name="psY", bufs=4, space="PSUM") as psY:
                # keep PE busy across the tail of the normalization chain
                tr1 = psY.tile([128, EMB], F32, tag="yps", name="warm1ps")
                for w in range(8):
                    nc.tensor.matmul(tr1[:, 0:QB], warm0[:, 0:128], warm0[:],
                                     start=True, stop=True)
                for s in range(NT):
                    y_ps = psY.tile([128, EMB], F32, tag="yps", name=f"yps{s}")
                    for nb in range(2):
                        for ct in range(2):
                            nc.tensor.matmul(
                                y_ps[:, nb * QB:(nb + 1) * QB],
                                qT[:, ct, s * 128:(s + 1) * 128],
                                wo_sb[:, ct, nb * QB:(nb + 1) * QB],
                                start=(ct == 0), stop=(ct == 1))
                    y_sb = ypool.tile([128, EMB], y_dt, tag="ysb",
                                      name=f"ysb{s}")
                    cp = nc.scalar.copy if s % 2 else nc.vector.tensor_copy
                    cp(y_sb[:], y_ps[:])
                    nc.sync.dma_start(y[s * 128:(s + 1) * 128, :], y_sb[:])

    nc.compile()
    return nc


def get_nc():
    global _NC
    if _NC is None:
        _NC = _build()
    return _NC


def make_in_maps(query, key, value, Wq, Wk, Wv, Wo):
    import ml_dtypes
    np_dt = {F32R: np.float32, BF16: ml_dtypes.bfloat16,
             FP16: np.float16}[MM]
    query = np.asarray(query, dtype=np.float32)
    key = np.asarray(key, dtype=np.float32)
    value = np.asarray(value, dtype=np.float32)
    Wq = np.asarray(Wq, dtype=np.float32)
    Wk = np.asarray(Wk, dtype=np.float32)
    Wv = np.asarray(Wv, dtype=np.float32)
    Wo = np.asarray(Wo, dtype=np.float32)
    xt = {(n, b): np.ascontiguousarray(x[b].T).astype(np_dt)
          for n, x in (("q", query), ("k", key), ("v", value))
          for b in range(B)}

    def warr(wt):
        # [G*128, N] -> [128, G*N]: partition-contiguous so the device
        # load is 128 contiguous descriptors
        a = wt.reshape(-1, 128, wt.shape[1]).transpose(1, 0, 2)
        return np.ascontiguousarray(a.reshape(128, -1)).astype(np_dt)

    in_maps = []
    for c in range(8):
        b, g = divmod(c, 4)
        hs = slice(g * CHD, (g + 1) * CHD)
        in_maps.append({
            "xq_t": xt[("q", b)],
            "xk_t": xt[("k", b)],
            "xv_t": xt[("v", b)],
            "wq_t": warr(Wq[hs, :].T),
            "wk_t": warr(Wk[hs, :].T),
            "wv_t": warr(Wv[hs, :].T),
            "wo_t": warr(Wo[:, hs].T),
        })
    return in_maps


def gather(results):
    out = np.zeros((B, S, EMB), dtype=np.float32)
    for c in range(8):
        out[c // 4] += results[c]["y"].astype(np.float32)
    return out


def kernel(**inputs) -> np.ndarray:
    nc = get_nc()
    in_maps = make_in_maps(**inputs)
    res = run_bass_kernel_spmd(nc, in_maps, core_ids=list(range(8)))
    return gather(res.results)


# revision 33
# speedup vs baseline: 1.6519x; 1.0172x over previous
"""Multi-head attention (B=2, S=2048, EMB=1024, H=16, hd=64) on 8 TRN2 cores.

Sharding: core c -> batch b = c//4, head-group g = c%4 (4 heads, 256 emb dims).
Per core (all matmuls bf16: full-rate 1 cyc/row streaming):
  A) Q^T = Wq_g @ x_b^T   [256, 2048]   (transposed layout, contraction on emb)
     K^T = Wk_g @ x_b^T   [256, 2048]
     V   = x_b @ Wv_g^T   [2048, 256]   (natural layout, +ones column per head)
  B) per head h: S^T[k,q] = K_h @ Q_h^T (16 k-tiles x [128, 2048] psum)
     P^T = exp(S^T/8): split between ACT (native Exp) and DVE (custom 2-op
     chain: deg-4 poly p~exp(s/64) then p^8) so the softmax isn't ACT-bound;
     U_aug[65, 2048] += [V_h|1].T @ P^T  (row 64 = softmax sums)
  C) r = 1/sums (DVE recip approx); broadcast r over 64 partitions via
     indicator matmul; O^T = U^T * r on GpSimd (written over the Q^T buffer)
  D) y = O @ Wo_g^T partial [2048, 1024]; host sums the 4 head-group partials.
"""
import numpy as np

import concourse.bass as bass
import concourse.tile as tile
from concourse import bacc, mybir
from concourse.bass_utils import run_bass_kernel_spmd

import os

F32 = mybir.dt.float32
F32R = mybir.dt.float32r
BF16 = mybir.dt.bfloat16
FP16 = mybir.dt.float16
# matmul dtype: bf16 (1 cyc/row, ~6e-3) | fp16 (2 cyc/row, ~1e-3) | f32r
MM_DT_NAME = os.environ.get("MM_DT", "bf16")
MM = {"f32r": F32R, "bf16": BF16, "fp16": FP16}[MM_DT_NAME]
IN_DT = {"f32r": F32, "bf16": BF16, "fp16": FP16}[MM_DT_NAME]
EXP = mybir.ActivationFunctionType.Exp
MULT = mybir.AluOpType.mult

EMB = 1024
S = 2048
B = 2
HG = 4           # heads per core
HD = 64
CHD = HG * HD    # 256 emb dims per core
ET = EMB // 128  # 8 e-tiles
NT = S // 128    # 16 s/k-tiles
QB = 512
NQB = S // QB    # 4

_NC = None

# ---- custom DVE exp: p(s) ~ exp(s/64) (deg-4, a0=1), then p^8 -------------
# minimax-with-a0=1 coefficients for exp(u) on u in [-0.8, 0.8], folded with
# the 1/64 argument scale (b_i = a_i / 64^i).
_A = (0.99930331, 0.49979974, 0.17207327, 0.04243063)
EXP_B1 = _A[0] / 64.0
EXP_B2 = _A[1] / 64.0 ** 2
EXP_B3 = _A[2] / 64.0 ** 3
EXP_B4 = _A[3] / 64.0 ** 4

# exp-tile engine assignment: scores psum is split into [128, 512] j-half
# tiles (1 PSUM bank each, bufs=2) so the j-halves double-buffer each other
# and the exp engines never gate the next scores matmul. Balance ACT vs DVE
# by throughput: A-stream on ACT; 10 of 16 B-stream t's on DVE.
B_DVE_T = frozenset((1, 2, 4, 5, 7, 8, 10, 11, 13, 14))
A_DVE_T = frozenset()


def _use_dve(stream, t, j):
    if stream == "A":
        return t in A_DVE_T
    # one extra DVE j-half per (mh, qh) evens the measured ACT/DVE load
    return t in B_DVE_T or (t == 0 and j == 0)


def _register_dve_op(name, spec, subdim=False):
    import concourse.dve_ops as dvo
    from concourse.dve_uop import DveOpSpec
    from concourse.dve_spec import lower
    from concourse.dve_spec import _has_src1 as has_src1

    for op in dvo.OPS:
        if op.name == name:
            return op
    opcode = dvo._CUSTOM_DVE_ROW_BASE + len(dvo.OPS)
    assert opcode < 0x20
    dvo._SUB_OPCODE_FOR_NAME[name] = opcode
    shas = {}
    for ver in ("v3", "v4"):
        tmp = DveOpSpec(name=name, opcode=opcode, uops=lower(spec, ver=ver),
                        rd1_en=has_src1(spec))
        shas[ver] = tmp.sha(ver)
    op = dvo.DveOp(name, spec, subdim=subdim, uops_sha=shas)
    dvo.OPS.append(op)
    dvo.CUSTOM_DVE_SPECS[name] = spec
    return op


def _make_exp_ops():
    from concourse.dve_spec import (
        Spec, Src0, C0, C1, C2, C3, One, sq, _spill_c3_to_src1,
    )

    u = Src0
    body = _spill_c3_to_src1(((((u * C0 + C1) * u + C2) * u + C3) * u) + One)

    def _ref_poly(in0, in1, s0, s1, imm2):
        return ((((in0 * s0 + s1) * in0 + imm2) * in0 + in1) * in0
                + np.float32(1.0)).astype(np.float32)

    poly = _register_dve_op("EXP_POLY4_ANT", Spec(body=body, reference=_ref_poly))

    def _ref_pow8(in0, in1, s0, s1, imm2):
        q = (in0 * in0).astype(np.float32)
        q = (q * q).astype(np.float32)
        return (q * q).astype(np.float32)

    pow8 = _register_dve_op("POW8_ANT",
                            Spec(body=sq(sq(sq(Src0))), reference=_ref_pow8))
    return poly, pow8


EXP_POLY4, POW8 = _make_exp_ops()


def _mm(ap):
    """View a dram input AP with the matmul dtype (bitcast only for f32r)."""
    return ap.bitcast(F32R) if MM == F32R else ap


def _build():
    nc = bacc.Bacc("TRN2", target_bir_lowering=False, debug=False)
    xq_t = nc.dram_tensor("xq_t", [EMB, S], IN_DT, kind="ExternalInput").ap()
    xk_t = nc.dram_tensor("xk_t", [EMB, S], IN_DT, kind="ExternalInput").ap()
    xv_t = nc.dram_tensor("xv_t", [EMB, S], IN_DT, kind="ExternalInput").ap()
    # weights pre-arranged on host to [128, ET*CHD] / [128, 2*EMB] so the
    # load is 128 contiguous 4KB descriptors instead of 1024 strided ones
    wq_t = nc.dram_tensor("wq_t", [128, ET * CHD], IN_DT,
                          kind="ExternalInput").ap()
    wk_t = nc.dram_tensor("wk_t", [128, ET * CHD], IN_DT,
                          kind="ExternalInput").ap()
    wv_t = nc.dram_tensor("wv_t", [128, ET * CHD], IN_DT,
                          kind="ExternalInput").ap()
    wo_t = nc.dram_tensor("wo_t", [128, 2 * EMB], IN_DT,
                          kind="ExternalInput").ap()
    # partial outputs leave in the matmul dtype: halves the output-DMA
    # drain at the kernel tail; the host gather accumulates in fp32
    y_dt = F32 if MM == F32R else MM
    y = nc.dram_tensor("y", [S, EMB], y_dt, kind="ExternalOutput").ap()

    with tile.TileContext(nc) as tc:
        with tc.tile_pool(name="const", bufs=1) as cpool, \
             tc.tile_pool(name="wqk", bufs=2) as wpool, \
             tc.tile_pool(name="big", bufs=1) as big, \
             tc.tile_pool(name="usb", bufs=4) as usb, \
             tc.tile_pool(name="xp", bufs=8) as xp, \
             tc.tile_pool(name="pt", bufs=4) as ptp, \
             tc.tile_pool(name="esc", bufs=3) as escp, \
             tc.tile_pool(name="yp", bufs=2) as ypool, \
             tc.tile_pool(name="rp", bufs=2) as rpool, \
             tc.tile_pool(name="rd", bufs=4, space="DRAM") as rdram:

            # ---- static weights (wo DMA deferred past phase A) ----
            wo_sb = cpool.tile([128, 2, EMB], MM, name="wo_sb")
            b1c = cpool.tile([128, 1], F32, name="b1c")
            nc.vector.memset(b1c[:], EXP_B1)

            qT = big.tile([128, 2, S], MM, name="qT")     # later reused as O^T
            kT = big.tile([128, 2, S], MM, name="kT")
            v_sb = big.tile([128, NT, HG * (HD + 1)], MM, name="v_sb")
            if MM == F32R:
                nc.vector.memset(v_sb[:].bitcast(F32), 1.0)
            else:
                nc.vector.memset(v_sb[:], 1.0)     # ones cols survive

            # ---- phase A: projections ----
            warm0 = cpool.tile([128, QB], MM, name="warm0")
            nc.vector.memset(warm0[:], 1.0)
            with tc.tile_pool(name="psA", bufs=8, space="PSUM") as psA:
                # Q^T and K^T: out[m, q] accumulated over e; m-halves use
                # 4 psum banks each so one half's copies overlap the other
                # half's matmuls
                for name, xdram, wdram, dst in (
                        ("q", xq_t, wq_t, qT), ("k", xk_t, wk_t, kT)):
                    w_sb = wpool.tile([128, ET, CHD], MM, tag="w",
                                      name=f"w{name}_sb")
                    nc.sync.dma_start(
                        w_sb[:].rearrange("pi po m -> pi (po m)"),
                        _mm(wdram))
                    xts = []
                    for e in range(ET):
                        x_t = xp.tile([128, S], MM, tag="x", name=f"x_{name}{e}")
                        nc.sync.dma_start(
                            x_t[:], _mm(xdram)[e * 128:(e + 1) * 128, :])
                        xts.append(x_t)
                    for m in range(2):
                        pss = [psA.tile([128, QB], F32, tag="ps",
                                        name=f"ps_{name}{m}{i}")
                               for i in range(NQB)]
                        for e in range(ET):
                            for qb in range(NQB):
                                nc.tensor.matmul(
                                    pss[qb][:],
                                    w_sb[:, e, m * 128:(m + 1) * 128],
                                    xts[e][:, qb * QB:(qb + 1) * QB],
                                    start=(e == 0), stop=(e == ET - 1))
                        for qb in range(NQB):
                            cp = nc.scalar.copy if qb % 2 else \
                                nc.vector.tensor_copy
                            cp(dst[:, m, qb * QB:(qb + 1) * QB],
                               pss[qb][:])

                # V natural: s-outer; all 8 xv e-tiles stay resident
                # so each s-tile owns one psum accumulation group.
                wv_sb = wpool.tile([128, ET, CHD], MM, tag="w", name="wv_sb")
                nc.sync.dma_start(
                    wv_sb[:].rearrange("pi po m -> pi (po m)"),
                    _mm(wv_t))
                xv_tiles = []
                for e in range(ET):
                    x_t = xp.tile([128, S], MM, tag="x", name=f"x_v{e}")
                    nc.sync.dma_start(
                        x_t[:], _mm(xv_t)[e * 128:(e + 1) * 128, :])
                    xv_tiles.append(x_t)
                # deferred weight loads ride behind the xv DMAs
                nc.sync.dma_start(
                    wo_sb[:].rearrange("p ct n -> p (ct n)"), _mm(wo_t))
                for s in range(NT):
                    v_ps = psA.tile([128, CHD], F32, tag="ps", name=f"ps_v{s}")
                    for e in range(ET):
                        nc.tensor.matmul(
                            v_ps[:], xv_tiles[e][:, s * 128:(s + 1) * 128],
                            wv_sb[:, e, :],
                            start=(e == 0), stop=(e == ET - 1))
                    src = v_ps[:].rearrange("p (h d) -> p h d", d=HD)
                    dst = v_sb[:, s, :].rearrange("p (h d) -> p h d",
                                                  d=HD + 1)[:, :, 0:HD]
                    cp = nc.scalar.copy if s % 2 else nc.vector.tensor_copy
                    cp(dst, src)

            # ---- phase B: attention, head-PAIRS packed on PE ----
            # Heads 2mh (rows 0-63) and 2mh+1 (rows 64-127) issue scores
            # matmuls into different PE row-groups + different psum banks, so
            # they run concurrently. q is split in halves so both heads'
            # U accumulators fit PSUM ([65, 1024] = 2 banks each).
            u_list = [None] * HG
            with tc.tile_pool(name="psS", bufs=2, space="PSUM") as psS, \
                 tc.tile_pool(name="psU", bufs=1, space="PSUM") as psU:
                # PE warm-up: dense dummy matmuls so the HAM clock gate sits
                # at K=8/8 entering the latency-sensitive B phase.
                trash = psS.tile([128, QB], F32, tag="spsA", name="warm")
                for w in range(24):
                    nc.tensor.matmul(
                        trash[:],
                        v_sb[:, 0, 0:128],
                        v_sb[:, 0:2, 0:256],
                        start=True, stop=True)
                SH = S // 2
                for mh in range(2):
                    hA, hB = 2 * mh, 2 * mh + 1
                    uA = usb.tile([HD + 1, S], F32, tag="u", name=f"u{hA}")
                    uB = usb.tile([HD + 1, S], F32, tag="u", name=f"u{hB}")
                    u_list[hA], u_list[hB] = uA, uB
                    for qh in range(2):
                        qo = qh * SH
                        uaccA = psU.tile([HD + 1, SH], F32, tag="uaccA",
                                         name=f"uaccA{mh}_{qh}")
                        uaccB = psU.tile([HD + 1, SH], F32, tag="uaccB",
                                         name=f"uaccB{mh}_{qh}")

                        def pv(t, p_pair):
                            # PV matmuls for tile t (emitted PV_LAG iters
                            # after its exp so the DVE chain latency never
                            # blocks the in-order PE queue)
                            for h2, uacc, p_t in ((hA, uaccA, p_pair[0]),
                                                  (hB, uaccB, p_pair[1])):
                                for j in range(2):
                                    nc.tensor.matmul(
                                        uacc[:, j * QB:(j + 1) * QB],
                                        v_sb[:, t,
                                             h2 * (HD + 1):(h2 + 1) * (HD + 1)],
                                        p_t[:, j * QB:(j + 1) * QB],
                                        start=(t == 0), stop=(t == NT - 1))

                        PV_LAG = 3
                        pq = []
                        for t in range(NT):
                            pA = ptp.tile([128, SH], MM, tag="ptA",
                                          name=f"ptA{mh}{qh}{t}")
                            pB = ptp.tile([128, SH], MM, tag="ptB",
                                          name=f"ptB{mh}{qh}{t}")
                            for j in range(2):
                                js = slice(j * QB, (j + 1) * QB)
                                sps = []
                                for bp, st in ((0, "A"), (64, "B")):
                                    sp = psS.tile([128, QB], F32,
                                                  tag=f"sps{st}",
                                                  name=f"sps{st}{mh}{qh}{t}{j}")
                                    nc.tensor.matmul(
                                        sp[:],
                                        kT[bp:bp + HD, mh,
                                           t * 128:(t + 1) * 128],
                                        qT[bp:bp + HD, mh,
                                           qo + j * QB:qo + (j + 1) * QB],
                                        start=True, stop=True)
                                    sps.append(sp)
                                for sp, p_t, dve in (
                                        (sps[0], pA, _use_dve("A", t, j)),
                                        (sps[1], pB, _use_dve("B", t, j))):
                                    if dve:
                                        sc = escp.tile(
                                            [128, QB], F32, tag="esc",
                                            name=f"esc{mh}{qh}{t}{j}")
                                        nc.vector._custom_dve(
                                            EXP_POLY4, out=sc[:], in0=sp[:],
                                            in1=b1c[:], s0=EXP_B4, s1=EXP_B3,
                                            imm2=EXP_B2)
                                        nc.vector._custom_dve(
                                            POW8, out=p_t[:, js], in0=sc[:])
                                    else:
                                        nc.scalar.activation(
                                            p_t[:, js], sp[:], EXP,
                                            scale=0.125)
                            pq.append((pA, pB))
                            if t >= PV_LAG:
                                pv(t - PV_LAG, pq[t - PV_LAG])
                        for t in range(NT - PV_LAG, NT):
                            pv(t, pq[t])
                        nc.vector.tensor_copy(uA[:, qo:qo + SH], uaccA[:])
                        nc.scalar.copy(uB[:, qo:qo + SH], uaccB[:])
                        # softmax normalization for this (pair, q-half):
                        # r = 1/sums (row 64), broadcast r across the 64
                        # partitions via a DRAM-bounce DMA (stride-0
                        # partition reads are legal from DRAM), O^T = U * r
                        # overwrites qT. All halves except the last overlap
                        # later B work (Pool); the last exposed half goes to
                        # DVE which is free by then. Doing this per q-half
                        # lets phase D's first s-tiles start right at B end,
                        # which also keeps the PE HAM-warm through the tail.
                        rp2 = rpool.tile([2, SH], F32, tag="rh",
                                         name=f"rp2_{mh}{qh}")
                        nc.sync.dma_start(rp2[0:1, :],
                                          uA[HD:HD + 1, qo:qo + SH])
                        nc.sync.dma_start(rp2[1:2, :],
                                          uB[HD:HD + 1, qo:qo + SH])
                        r2 = rpool.tile([2, SH], F32, tag="rh2",
                                        name=f"r2_{mh}{qh}")
                        nc.vector.reciprocal_approx_fast(out=r2[:], in_=rp2[:])
                        last = (mh == 1 and qh == 1)
                        for h2, u_h in ((hA, uA), (hB, uB)):
                            bp2 = 64 * (h2 % 2)
                            rd = rdram.tile([1, SH], F32, name=f"rd{h2}_{qh}")
                            nc.sync.dma_start(rd[:],
                                              r2[h2 % 2:h2 % 2 + 1, :])
                            rb = rpool.tile([HD, SH], F32, tag="rb",
                                            name=f"rb{h2}_{qh}", bufs=3)
                            nc.sync.dma_start(rb[:],
                                              rd[:].to_broadcast([HD, SH]))
                            eng = nc.vector if last else nc.gpsimd
                            eng.tensor_tensor(
                                qT[bp2:bp2 + HD, mh, qo:qo + SH],
                                u_h[0:HD, qo:qo + SH],
                                rb[:], MULT)

            # ---- phase D: output projection (qT now holds O^T) ----
            with tc.tile_pool(name="psY", bufs=4, space="PSUM") as psY:
                # keep PE busy across the tail of the normalization chain
                tr1 = psY.tile([128, EMB], F32, tag="yps", name="warm1ps")
                for w in range(8):
                    nc.tensor.matmul(tr1[:, 0:QB], warm0[:, 0:128], warm0[:],
                                     start=True, stop=True)
                for s in range(NT):
                    y_ps = psY.tile([128, EMB], F32, tag="yps", name=f"yps{s}")
                    for nb in range(2):
                        for ct in range(2):
                            nc.tensor.matmul(
                                y_ps[:, nb * QB:(nb + 1) * QB],
                                qT[:, ct, s * 128:(s + 1) * 128],
                                wo_sb[:, ct, nb * QB:(nb + 1) * QB],
                                start=(ct == 0), stop=(ct == 1))
                    y_sb = ypool.tile([128, EMB], y_dt, tag="ysb",
                                      name=f"ysb{s}")
                    cp = nc.scalar.copy if s % 2 else nc.vector.tensor_copy
                    cp(y_sb[:], y_ps[:])
                    nc.sync.dma_start(y[s * 128:(s + 1) * 128, :], y_sb[:])

    nc.compile()
    return nc


def get_nc():
    global _NC
    if _NC is None:
        _NC = _build()
    return _NC


def make_in_maps(query, key, value, Wq, Wk, Wv, Wo):
    import ml_dtypes
    np_dt = {F32R: np.float32, BF16: ml_dtypes.bfloat16,
             FP16: np.float16}[MM]
    query = np.asarray(query, dtype=np.float32)
    key = np.asarray(key, dtype=np.float32)
    value = np.asarray(value, dtype=np.float32)
    Wq = np.asarray(Wq, dtype=np.float32)
    Wk = np.asarray(Wk, dtype=np.float32)
    Wv = np.asarray(Wv, dtype=np.float32)
    Wo = np.asarray(Wo, dtype=np.float32)
    xt = {(n, b): np.ascontiguousarray(x[b].T).astype(np_dt)
          for n, x in (("q", query), ("k", key), ("v", value))
          for b in range(B)}

    def warr(wt):
        # [G*128, N] -> [128, G*N]: partition-contiguous so the device
        # load is 128 contiguous descriptors
        a = wt.reshape(-1, 128, wt.shape[1]).transpose(1, 0, 2)
        return np.ascontiguousarray(a.reshape(128, -1)).astype(np_dt)

    in_maps = []
    for c in range(8):
        b, g = divmod(c, 4)
        hs = slice(g * CHD, (g + 1) * CHD)
        in_maps.append({
            "xq_t": xt[("q", b)],
            "xk_t": xt[("k", b)],
            "xv_t": xt[("v", b)],
            "wq_t": warr(Wq[hs, :].T),
            "wk_t": warr(Wk[hs, :].T),
            "wv_t": warr(Wv[hs, :].T),
            "wo_t": warr(Wo[:, hs].T),
        })
    return in_maps


def gather(results):
    out = np.zeros((B, S, EMB), dtype=np.float32)
    for c in range(8):
        out[c // 4] += results[c]["y"].astype(np.float32)
    return out


def kernel(**inputs) -> np.ndarray:
    nc = get_nc()
    in_maps = make_in_maps(**inputs)
    res = run_bass_kernel_spmd(nc, in_maps, core_ids=list(range(8)))
    return gather(res.results)
